# revision 41
# baseline (speedup 1.0000x reference)
"""Tensor-parallel GQA multi-head-attention kernel for 8 trn2 NeuronCores.

Problem: B=2, T=2048, D=2048, H=16 q-heads, KV=4 kv-heads, HD=128,
causal attention with interleaved RoPE, y = attn_out @ Wo.

Sharding (tensor-parallel over heads, per the hint):
  core c = b*4 + g   (b = batch index, g = kv-head / q-head-group index)
  Each core computes q-heads 4g..4g+3 and kv-head g for batch b, plus the
  partial output  y_partial = attn_heads @ Wo[rows of those heads]  (row-
  parallel Wo).  The host sums the 4 partials per batch (the unshard of the
  row-parallel all-reduce) and stacks the 2 batches.

On-chip design (per core, bf16 data / f32 PSUM+softmax):
  - host repacks every input into [128, ...] arrays so each load is ONE
    wide dma_start (16KB/partition rows -> 16KB DMA descriptors); x and
    the rope tables are loaded per 512-col chunk so compute starts after
    ~3MB instead of after the full 13MB.
  - projections: q^T[h] = Wq_h^T @ xT per 512-col chunk (PSUM k-accum),
    RoPE'd via half-swap (SBUF-SBUF DMA) + 3 bf16 DVE ops into qT/kT;
    v natural via lhsT = xT tile, 4 T-tiles packed into one PSUM bank.
  - attention per (head, 512-col q chunk): for each 128-row k tile,
    S^T = kT_tile.T @ qT chunk -> PSUM [128,512]; diagonal blocks get a
    -30000 mask add (DVE); ACT computes P = exp(scale*S^T) -> SBUF bf16;
    PV accumulates out^T[HD,512] in PSUM.  Softmax denominators: P tiles
    are accumulated on DVE (bf16) into Pacc, then ONE [128,128]-ones
    matmul broadcasts column sums -> reciprocal -> one DVE multiply.
    Fully-masked (future) blocks are skipped everywhere.
  - Wo: y tile [128,512] = sum_h attnT_h.T @ Wo_h chunk (PSUM), ACT copy
    into a [128, 2048] bf16 staging tile, one dma_start per 128-row block.
  - emission interleaves attention of chunk c with projections of chunk
    c+2 and Wo of chunk c-1 so the PE queue always has independent work
    while exp/rope chains resolve.
"""

import math
import sys

import numpy as np

for _p in ("/opt/trn_rl_repo", "/root/.axon_site",
           "/root/.axon_site/_ro/trn_rl_repo",
           "/root/.axon_site/_ro/pypackages"):
    if _p not in sys.path:
        sys.path.append(_p)

B, T, D = 2, 2048, 2048
H, KV, HD = 16, 4, 128
ROPE_BASE = 10000.0
N_CORES = 8
HPC = 4                  # q heads per core
DQ = HPC * HD            # 512 q-dims per core
SCALE = 1.0 / math.sqrt(HD)
MASK_VAL = -30000.0

_CACHE = {}


def _build_nc(t_len=T):
    """Build the single-core SPMD Bass/Tile program (cached)."""
    import concourse.bass as bass
    import concourse.mybir as mybir
    import concourse.tile as tile
    from concourse import bacc

    f32 = mybir.dt.float32
    bf16 = mybir.dt.bfloat16
    ts = bass.ts

    NT = t_len // 128        # number of 128-row T tiles
    NK = D // 128            # contraction chunks for projections
    NCQ = t_len // 512       # number of 512-wide q chunks

    nc = bacc.Bacc("TRN2", target_bir_lowering=False, debug=False,
                   num_devices=N_CORES)

    x_d = nc.dram_tensor("xp", [128, NCQ, NK, 512], bf16,
                         kind="ExternalInput").ap()
    wq_d = nc.dram_tensor("wq", [128, HPC, NK, 128], bf16,
                          kind="ExternalInput").ap()
    wk_d = nc.dram_tensor("wk", [128, NK, 128], bf16,
                          kind="ExternalInput").ap()
    wv_d = nc.dram_tensor("wv", [128, NK, 128], bf16,
                          kind="ExternalInput").ap()
    wo_d = nc.dram_tensor("wo", [128, HPC, D], bf16,
                          kind="ExternalInput").ap()
    tab_d = nc.dram_tensor("tab", [128, NCQ, 2, 512], bf16,
                           kind="ExternalInput").ap()
    mask_d = nc.dram_tensor("mask", [128, 128], bf16,
                            kind="ExternalInput").ap()
    id_d = nc.dram_tensor("ident", [128, 128], f32,
                          kind="ExternalInput").ap()
    y_d = nc.dram_tensor("y", [t_len, D], bf16, kind="ExternalOutput").ap()

    Exp = mybir.ActivationFunctionType.Exp

    with tile.TileContext(nc) as tc:
        with (
            tc.tile_pool(name="const", bufs=1) as const,
            tc.tile_pool(name="qkv", bufs=1) as qkv,
            tc.tile_pool(name="attn", bufs=3) as attn_pool,
            tc.tile_pool(name="p", bufs=8) as p_pool,
            tc.tile_pool(name="rope", bufs=2) as rope_pool,
            tc.tile_pool(name="pacc", bufs=2) as pacc_pool,
            tc.tile_pool(name="recip", bufs=2) as recip_pool,
            tc.tile_pool(name="y", bufs=2) as y_pool,
            tc.tile_pool(name="psum", bufs=1, space="PSUM") as psum,
        ):
            # ---- SBUF input tiles ----
            x_sb = const.tile([128, NCQ, NK, 512], bf16, tag="x")
            wq_sb = const.tile([128, HPC, NK, 128], bf16, tag="wq")
            wk_sb = const.tile([128, NK, 128], bf16, tag="wk")
            wv_sb = const.tile([128, NK, 128], bf16, tag="wv")
            wo_sb = const.tile([128, HPC, D], bf16, tag="wo")
            tab_sb = const.tile([128, NCQ, 2, 512], bf16, tag="tab")
            mask_sb = const.tile([128, 128], bf16, tag="mask")
            id_sb = const.tile([128, 128], f32, tag="ident")
            ones_sb = const.tile([128, 128], bf16, tag="ones")

            # ---- input loads: few wide DMAs, ordered so chunk-0 compute
            # starts as early as possible ----
            # upfront loads: only what chunks 0/1 need.  x2/x3/wo/tab2/tab3
            # are issued later from the gpsimd queue, sequenced behind the
            # rope swap DMAs so those never starve for DMA-engine bandwidth.
            nc.sync.dma_start(wk_sb[:], wk_d[:])
            nc.sync.dma_start(x_sb[:, 0, 0:8, :], x_d[:, 0, 0:8, :])
            nc.sync.dma_start(x_sb[:, 0, 8:NK, :], x_d[:, 0, 8:NK, :])
            nc.sync.dma_start(wq_sb[:, 0], wq_d[:, 0])
            nc.sync.dma_start(tab_sb[:, 0], tab_d[:, 0])
            for h in range(1, HPC):
                nc.sync.dma_start(wq_sb[:, h], wq_d[:, h])
            nc.sync.dma_start(wv_sb[:], wv_d[:])
            nc.sync.dma_start(mask_sb[:], mask_d[:])
            nc.sync.dma_start(id_sb[:], id_d[:])
            nc.sync.dma_start(x_sb[:, 1], x_d[:, 1])
            nc.sync.dma_start(tab_sb[:, 1], tab_d[:, 1])
            nc.vector.memset(ones_sb[:], 1.0)

            # PE warm-up: serial matmuls on constants during the load window
            # ramp the tensor-engine clock to full speed before real work.
            warm_rhs = const.tile([128, 512], bf16, tag="warm")
            nc.vector.memset(warm_rhs[:], 0.0)
            warm_ps = psum.tile([128, 512], f32, tag="proj", bufs=2)
            for _ in range(21):
                nc.tensor.matmul(warm_ps[:], ones_sb[:], warm_rhs[:],
                                 start=True, stop=True)

            # per-chunk activations (separate tiles keep cross-chunk
            # dependencies precise in the Tile framework)
            qTs = [qkv.tile([128, HPC, 512], bf16, tag=f"qT{c}",
                            name=f"qT{c}") for c in range(NCQ)]
            kTs = [qkv.tile([128, 512], bf16, tag=f"kT{c}",
                            name=f"kT{c}") for c in range(NCQ)]
            vs = [qkv.tile([128, 512], bf16, tag=f"v{c}",
                           name=f"v{c}") for c in range(NCQ)]

            def rope_apply(dst, f, s, c):
                """dst = f*cos + s*ssig for one [128,512] head chunk."""
                tm = rope_pool.tile([128, 512], bf16, tag="tm")
                nc.vector.tensor_mul(dst, f, tab_sb[:, c, 0, :])
                nc.vector.tensor_mul(tm[:], s, tab_sb[:, c, 1, :])
                nc.vector.tensor_add(dst, dst, tm[:])

            def u_kproj(c):
                kp = psum.tile([128, 512], f32, tag="proj", bufs=2)
                for k in range(NK):
                    nc.tensor.matmul(kp[:], wk_sb[:, k, :], x_sb[:, c, k, :],
                                     start=(k == 0), stop=(k == NK - 1))
                kf = rope_pool.tile([128, 512], bf16, tag="kf")
                ks = rope_pool.tile([128, 512], bf16, tag="ks")
                nc.vector.tensor_copy(kf[:], kp[:])
                nc.gpsimd.dma_start(ks[0:64, :], kf[64:128, :])
                nc.gpsimd.dma_start(ks[64:128, :], kf[0:64, :])
                rope_apply(kTs[c][:], kf[:], ks[:], c)

            def u_qproj(c, h, qf4):
                qp = psum.tile([128, 512], f32, tag="proj", bufs=2)
                for k in range(NK):
                    nc.tensor.matmul(qp[:], wq_sb[:, h, k, :],
                                     x_sb[:, c, k, :],
                                     start=(k == 0), stop=(k == NK - 1))
                nc.vector.tensor_copy(qf4[:, h, :], qp[:])

            def u_qrope(c, qf4):
                qs4 = rope_pool.tile([128, HPC, 512], bf16, tag="qs4")
                nc.gpsimd.dma_start(qs4[0:64, :, :], qf4[64:128, :, :])
                nc.gpsimd.dma_start(qs4[64:128, :, :], qf4[0:64, :, :])
                for h in range(HPC):
                    rope_apply(qTs[c][:, h, :], qf4[:, h, :], qs4[:, h, :], c)

            def u_vproj(c):
                """v projection for chunk c: v^T chunk then PE transpose."""
                vtp = psum.tile([128, 512], f32, tag="proj", bufs=2)
                for k in range(NK):
                    nc.tensor.matmul(vtp[:], wv_sb[:, k, :], x_sb[:, c, k, :],
                                     start=(k == 0), stop=(k == NK - 1))
                vt = rope_pool.tile([128, 512], f32, tag="vt")
                nc.vector.tensor_copy(vt[:], vtp[:])
                vtr = psum.tile([128, 512], f32, tag="proj", bufs=2)
                for tt in range(4):
                    nc.tensor.transpose(vtr[:, ts(tt, 128)],
                                        vt[:, ts(tt, 128)], id_sb[:])
                nc.vector.tensor_copy(vs[c][:], vtr[:])

            def u_attn_head(c, h, attn_t):
                nj = 4 * c + 4
                out_ps = psum.tile([128, 512], f32, tag="out", bufs=2)
                pacc = pacc_pool.tile([128, 512], bf16, tag="pacc")
                for j in range(nj):
                    o = j - 4 * c
                    lo = max(o, 0) * 128
                    s_ps = psum.tile([128, 512], f32, tag="s", bufs=3)
                    nc.tensor.matmul(s_ps[:, lo:],
                                     kTs[j // 4][:, ts(j % 4, 128)],
                                     qTs[c][:, h, lo:],
                                     start=True, stop=True)
                    if o >= 0:
                        nc.vector.tensor_add(s_ps[:, lo:lo + 128],
                                             s_ps[:, lo:lo + 128],
                                             mask_sb[:])
                    p = p_pool.tile([128, 512], bf16, tag="p")
                    nc.scalar.activation(p[:, lo:], s_ps[:, lo:], Exp,
                                         bias=0.0, scale=SCALE)
                    if j == 0:
                        nc.vector.tensor_copy(pacc[:], p[:])
                    else:
                        nc.vector.tensor_add(pacc[:, lo:], pacc[:, lo:],
                                             p[:, lo:])
                    nc.tensor.matmul(out_ps[:, lo:],
                                     vs[j // 4][:, ts(j % 4, 128)],
                                     p[:, lo:],
                                     start=(j == 0), stop=(j == nj - 1))
                sums_ps = psum.tile([128, 512], f32, tag="sums", bufs=1)
                nc.tensor.matmul(sums_ps[:], ones_sb[:], pacc[:],
                                 start=True, stop=True)
                rc = recip_pool.tile([128, 512], f32, tag="rc")
                nc.vector.reciprocal_approx_fast(out=rc[:], in_=sums_ps[:])
                nc.vector.tensor_mul(attn_t[:, h, :], out_ps[:], rc[:])

            def u_wo(c, tq, nn, attn_t, y_sb):
                yp = psum.tile([128, 512], f32, tag="s", bufs=3)
                for h in range(HPC):
                    nc.tensor.matmul(yp[:], attn_t[:, h, ts(tq, 128)],
                                     wo_sb[:, h, ts(nn, 512)],
                                     start=(h == 0), stop=(h == 3))
                nc.scalar.copy(y_sb[:, ts(nn, 512)], yp[:])
                row0 = (4 * c + tq) * 128
                if c == NCQ - 1 and tq == 3:
                    # last row-block: store per 512-col slice so the final
                    # DMA isn't serialized behind all four copies
                    nc.sync.dma_start(y_d[row0:row0 + 128, ts(nn, 512)],
                                      y_sb[:, ts(nn, 512)])
                elif nn == 3:
                    nc.sync.dma_start(y_d[row0:row0 + 128, :], y_sb[:])

            # ---- unit construction / schedule ----
            def u_late_loads(c):
                """Deferred bulk loads, issued on the gpsimd DMA queue so
                they sequence AFTER the rope-swap DMAs emitted earlier."""
                if c == 0:
                    nc.gpsimd.dma_start(x_sb[:, 2], x_d[:, 2])
                    nc.gpsimd.dma_start(tab_sb[:, 2], tab_d[:, 2])
                    nc.gpsimd.dma_start(wo_sb[:], wo_d[:])
                elif c == 1:
                    nc.gpsimd.dma_start(x_sb[:, 3], x_d[:, 3])
                    nc.gpsimd.dma_start(tab_sb[:, 3], tab_d[:, 3])

            def proj_units(c):
                qf4 = rope_pool.tile([128, HPC, 512], bf16, tag="qf4")
                units = [lambda c=c: u_kproj(c)]
                for h in range(HPC):
                    units.append(lambda c=c, h=h, q=qf4: u_qproj(c, h, q))
                units.append(lambda c=c, q=qf4: u_qrope(c, q))
                units.append(lambda c=c: u_vproj(c))
                if c <= 1:
                    units.append(lambda c=c: u_late_loads(c))
                return units

            def wo_units(c, attn_t):
                units = []
                for tq in range(4):
                    y_sb = y_pool.tile([128, D], bf16, tag="y")
                    for nn in range(4):
                        units.append(
                            lambda c=c, tq=tq, nn=nn, a=attn_t, y=y_sb:
                            u_wo(c, tq, nn, a, y))
                return units

            # chunks 0 and 1 projections up front (fills the load window)
            for u in proj_units(0) + proj_units(1):
                u()
            pending_wo = []
            for c in range(NCQ):
                # interleave next projections + previous chunk's Wo between
                # the attention heads of chunk c
                fillers = (proj_units(c + 2) if c + 2 < NCQ else [])
                fillers += pending_wo
                attn_t = attn_pool.tile([128, HPC, 512], bf16, tag="attnT")
                nf = len(fillers)
                for h in range(HPC):
                    u_attn_head(c, h, attn_t)
                    take = nf // 4 + (1 if h < nf % 4 else 0)
                    for _ in range(take):
                        fillers.pop(0)()
                for u in fillers:
                    u()
                pending_wo = wo_units(c, attn_t)
            for u in pending_wo:
                u()

    nc.finalize()
    return nc


def _prep_inputs(x, Wq, Wk, Wv, Wo, t_len=T):
    """Host-side shard + layout prep -> per-core input maps."""
    import ml_dtypes
    bf16 = ml_dtypes.bfloat16

    NK = D // 128
    NCQ = t_len // 512

    x = np.asarray(x, np.float32)
    Wq = np.asarray(Wq, np.float32)
    Wk = np.asarray(Wk, np.float32)
    Wv = np.asarray(Wv, np.float32)
    Wo = np.asarray(Wo, np.float32)

    # RoPE de-interleave permutation within one head: [evens | odds]
    perm = np.concatenate([np.arange(0, HD, 2), np.arange(1, HD, 2)])

    # rope tables (match reference: freqs = t * base**(-2j/HD))
    inv = 1.0 / (ROPE_BASE ** (np.arange(0, HD, 2, dtype=np.float32) / HD))
    tpos = np.arange(t_len, dtype=np.float32)
    f = inv[:, None] * tpos[None, :]                       # [64, T]
    cos_dup = np.concatenate([np.cos(f), np.cos(f)], 0)    # [128, T]
    ssig = np.concatenate([-np.sin(f), np.sin(f)], 0)      # [128, T]
    # tab[p, c, 0, :] = cos chunk c; tab[p, c, 1, :] = ssig chunk c
    tab = np.stack([cos_dup.reshape(128, NCQ, 512),
                    ssig.reshape(128, NCQ, 512)], axis=2).astype(bf16)

    # strict-lower-triangular causal mask template for the diagonal block
    r = np.arange(128)[:, None]
    col = np.arange(128)[None, :]
    mask_t = np.where(r > col, MASK_VAL, 0.0).astype(bf16)
    ident = np.eye(128, dtype=np.float32)
    jswap = np.roll(np.eye(128, dtype=np.float32), 64, axis=0).astype(bf16)

    in_maps = []
    for b in range(B):
        # xp[p, c, k, j] = x[b, c*512+j, k*128+p]
        xp = np.ascontiguousarray(
            x[b, :t_len].T.reshape(NK, 128, NCQ, 512)
            .transpose(1, 2, 0, 3)).astype(bf16)
        for g in range(KV):
            wq_g = Wq[:, g * DQ:(g + 1) * DQ].reshape(D, HPC, HD)
            wq_g = wq_g[:, :, perm]                       # [D, HPC, 128]
            wqp = np.ascontiguousarray(
                wq_g.reshape(NK, 128, HPC, 128)
                .transpose(1, 2, 0, 3)).astype(bf16)      # [p, h, k, j]
            wk_g = Wk[:, g * HD:(g + 1) * HD][:, perm]
            wkp = np.ascontiguousarray(
                wk_g.reshape(NK, 128, 128).transpose(1, 0, 2)).astype(bf16)
            wv_g = Wv[:, g * HD:(g + 1) * HD]
            wvp = np.ascontiguousarray(
                wv_g.reshape(NK, 128, 128).transpose(1, 0, 2)).astype(bf16)
            # wop[p, h, :] = Wo[(4g+h)*128 + p, :]
            wop = np.ascontiguousarray(
                Wo[g * DQ:(g + 1) * DQ, :].reshape(HPC, 128, D)
                .transpose(1, 0, 2)).astype(bf16)
            in_maps.append({
                "xp": xp, "wq": wqp, "wk": wkp, "wv": wvp,
                "wo": wop, "tab": tab, "mask": mask_t, "ident": ident,
                "jswap": jswap,
            })
    return in_maps


def run(inputs, trace=False, t_len=T):
    """Run the sharded kernel; returns (y_full, BassKernelResults)."""
    from concourse.bass_utils import run_bass_kernel_spmd

    key = ("nc", t_len)
    if key not in _CACHE:
        _CACHE[key] = _build_nc(t_len)
    nc = _CACHE[key]

    in_maps = _prep_inputs(inputs["x"], inputs["Wq"], inputs["Wk"],
                           inputs["Wv"], inputs["Wo"], t_len)
    res = run_bass_kernel_spmd(nc, in_maps, list(range(N_CORES)), trace=trace)

    y = np.empty((B, t_len, D), np.float32)
    for b in range(B):
        acc = np.zeros((t_len, D), np.float32)
        for g in range(KV):
            acc += np.asarray(res.results[b * KV + g]["y"], np.float32)
        y[b] = acc
    return y, res


def kernel(**inputs) -> np.ndarray:
    y, _ = run(inputs, trace=False)
    return y


# revision 43
# speedup vs baseline: 1.0090x; 1.0090x over previous
"""Tensor-parallel GQA multi-head-attention kernel for 8 trn2 NeuronCores.

Problem: B=2, T=2048, D=2048, H=16 q-heads, KV=4 kv-heads, HD=128,
causal attention with interleaved RoPE, y = attn_out @ Wo.

Sharding (tensor-parallel over heads, per the hint):
  core c = b*4 + g   (b = batch index, g = kv-head / q-head-group index)
  Each core computes q-heads 4g..4g+3 and kv-head g for batch b, plus the
  partial output  y_partial = attn_heads @ Wo[rows of those heads]  (row-
  parallel Wo).  The host sums the 4 partials per batch (the unshard of the
  row-parallel all-reduce) and stacks the 2 batches.

On-chip design (per core, bf16 data / f32 PSUM+softmax):
  - host repacks every input into [128, ...] arrays so each load is ONE
    wide dma_start (16KB/partition rows -> 16KB DMA descriptors); x and
    the rope tables are loaded per 512-col chunk so compute starts after
    ~3MB instead of after the full 13MB.
  - projections: q^T[h] = Wq_h^T @ xT per 512-col chunk (PSUM k-accum),
    RoPE'd via half-swap (SBUF-SBUF DMA) + 3 bf16 DVE ops into qT/kT;
    v natural via lhsT = xT tile, 4 T-tiles packed into one PSUM bank.
  - attention per (head, 512-col q chunk): for each 128-row k tile,
    S^T = kT_tile.T @ qT chunk -> PSUM [128,512]; diagonal blocks get a
    -30000 mask add (DVE); ACT computes P = exp(scale*S^T) -> SBUF bf16;
    PV accumulates out^T[HD,512] in PSUM.  Softmax denominators: P tiles
    are accumulated on DVE (bf16) into Pacc, then ONE [128,128]-ones
    matmul broadcasts column sums -> reciprocal -> one DVE multiply.
    Fully-masked (future) blocks are skipped everywhere.
  - Wo: y tile [128,512] = sum_h attnT_h.T @ Wo_h chunk (PSUM), ACT copy
    into a [128, 2048] bf16 staging tile, one dma_start per 128-row block.
  - emission interleaves attention of chunk c with projections of chunk
    c+2 and Wo of chunk c-1 so the PE queue always has independent work
    while exp/rope chains resolve.
"""

import math
import sys

import numpy as np

for _p in ("/opt/trn_rl_repo", "/root/.axon_site",
           "/root/.axon_site/_ro/trn_rl_repo",
           "/root/.axon_site/_ro/pypackages"):
    if _p not in sys.path:
        sys.path.append(_p)

B, T, D = 2, 2048, 2048
H, KV, HD = 16, 4, 128
ROPE_BASE = 10000.0
N_CORES = 8
HPC = 4                  # q heads per core
DQ = HPC * HD            # 512 q-dims per core
SCALE = 1.0 / math.sqrt(HD)
MASK_VAL = -30000.0

_CACHE = {}


def _build_nc(t_len=T):
    """Build the single-core SPMD Bass/Tile program (cached)."""
    import concourse.bass as bass
    import concourse.mybir as mybir
    import concourse.tile as tile
    from concourse import bacc

    f32 = mybir.dt.float32
    bf16 = mybir.dt.bfloat16
    ts = bass.ts

    NT = t_len // 128        # number of 128-row T tiles
    NK = D // 128            # contraction chunks for projections
    NCQ = t_len // 512       # number of 512-wide q chunks

    nc = bacc.Bacc("TRN2", target_bir_lowering=False, debug=False,
                   num_devices=N_CORES)

    x_d = nc.dram_tensor("xp", [128, NCQ, NK, 512], bf16,
                         kind="ExternalInput").ap()
    wq_d = nc.dram_tensor("wq", [128, HPC, NK, 128], bf16,
                          kind="ExternalInput").ap()
    wk_d = nc.dram_tensor("wk", [128, NK, 128], bf16,
                          kind="ExternalInput").ap()
    wv_d = nc.dram_tensor("wv", [128, NK, 128], bf16,
                          kind="ExternalInput").ap()
    wo_d = nc.dram_tensor("wo", [128, HPC, D], bf16,
                          kind="ExternalInput").ap()
    tab_d = nc.dram_tensor("tab", [128, NCQ, 2, 512], bf16,
                           kind="ExternalInput").ap()
    mask_d = nc.dram_tensor("mask", [128, 128], bf16,
                            kind="ExternalInput").ap()
    id_d = nc.dram_tensor("ident", [128, 128], f32,
                          kind="ExternalInput").ap()
    y_d = nc.dram_tensor("y", [t_len, D], bf16, kind="ExternalOutput").ap()

    Exp = mybir.ActivationFunctionType.Exp

    with tile.TileContext(nc) as tc:
        with (
            tc.tile_pool(name="const", bufs=1) as const,
            tc.tile_pool(name="qkv", bufs=1) as qkv,
            tc.tile_pool(name="attn", bufs=3) as attn_pool,
            tc.tile_pool(name="p", bufs=8) as p_pool,
            tc.tile_pool(name="rope", bufs=2) as rope_pool,
            tc.tile_pool(name="pacc", bufs=2) as pacc_pool,
            tc.tile_pool(name="recip", bufs=2) as recip_pool,
            tc.tile_pool(name="y", bufs=2) as y_pool,
            tc.tile_pool(name="psum", bufs=1, space="PSUM") as psum,
        ):
            # ---- SBUF input tiles ----
            x_sb = const.tile([128, NCQ, NK, 512], bf16, tag="x")
            wq_sb = const.tile([128, HPC, NK, 128], bf16, tag="wq")
            wk_sb = const.tile([128, NK, 128], bf16, tag="wk")
            wv_sb = const.tile([128, NK, 128], bf16, tag="wv")
            wo_sb = const.tile([128, HPC, D], bf16, tag="wo")
            tab_sb = const.tile([128, NCQ, 2, 512], bf16, tag="tab")
            mask_sb = const.tile([128, 128], bf16, tag="mask")
            id_sb = const.tile([128, 128], f32, tag="ident")
            ones_sb = const.tile([128, 128], bf16, tag="ones")

            # ---- input loads: few wide DMAs, ordered so chunk-0 compute
            # starts as early as possible ----
            # upfront loads: only what chunks 0/1 need.  x2/x3/wo/tab2/tab3
            # are issued later from the gpsimd queue, sequenced behind the
            # rope swap DMAs so those never starve for DMA-engine bandwidth.
            nc.sync.dma_start(wk_sb[:], wk_d[:])
            nc.sync.dma_start(x_sb[:, 0, 0:8, :], x_d[:, 0, 0:8, :])
            nc.sync.dma_start(x_sb[:, 0, 8:NK, :], x_d[:, 0, 8:NK, :])
            nc.sync.dma_start(wq_sb[:, 0], wq_d[:, 0])
            nc.sync.dma_start(tab_sb[:, 0], tab_d[:, 0])
            for h in range(1, HPC):
                nc.sync.dma_start(wq_sb[:, h], wq_d[:, h])
            nc.sync.dma_start(wv_sb[:], wv_d[:])
            nc.sync.dma_start(mask_sb[:], mask_d[:])
            nc.sync.dma_start(id_sb[:], id_d[:])
            nc.sync.dma_start(x_sb[:, 1], x_d[:, 1])
            nc.sync.dma_start(tab_sb[:, 1], tab_d[:, 1])
            nc.vector.memset(ones_sb[:], 1.0)

            # PE warm-up: serial matmuls on constants during the load window
            # ramp the tensor-engine clock to full speed before real work.
            warm_rhs = const.tile([128, 512], bf16, tag="warm")
            nc.vector.memset(warm_rhs[:], 0.0)
            warm_ps = psum.tile([128, 512], f32, tag="proj", bufs=2)
            for _ in range(24):
                nc.tensor.matmul(warm_ps[:], ones_sb[:], warm_rhs[:],
                                 start=True, stop=True)

            # per-chunk activations (separate tiles keep cross-chunk
            # dependencies precise in the Tile framework)
            qTs = [qkv.tile([128, HPC, 512], bf16, tag=f"qT{c}",
                            name=f"qT{c}") for c in range(NCQ)]
            kTs = [qkv.tile([128, 512], bf16, tag=f"kT{c}",
                            name=f"kT{c}") for c in range(NCQ)]
            vs = [qkv.tile([128, 512], bf16, tag=f"v{c}",
                           name=f"v{c}") for c in range(NCQ)]

            def rope_apply(dst, f, s, c):
                """dst = f*cos + s*ssig for one [128,512] head chunk."""
                tm = rope_pool.tile([128, 512], bf16, tag="tm")
                nc.vector.tensor_mul(dst, f, tab_sb[:, c, 0, :])
                nc.vector.tensor_mul(tm[:], s, tab_sb[:, c, 1, :])
                nc.vector.tensor_add(dst, dst, tm[:])

            def u_kproj(c):
                kp = psum.tile([128, 512], f32, tag="proj", bufs=2)
                for k in range(NK):
                    nc.tensor.matmul(kp[:], wk_sb[:, k, :], x_sb[:, c, k, :],
                                     start=(k == 0), stop=(k == NK - 1))
                kf = rope_pool.tile([128, 512], bf16, tag="kf")
                ks = rope_pool.tile([128, 512], bf16, tag="ks")
                nc.vector.tensor_copy(kf[:], kp[:])
                nc.gpsimd.dma_start(ks[0:64, :], kf[64:128, :])
                nc.gpsimd.dma_start(ks[64:128, :], kf[0:64, :])
                rope_apply(kTs[c][:], kf[:], ks[:], c)

            def u_qproj(c, h, qf4):
                qp = psum.tile([128, 512], f32, tag="proj", bufs=2)
                for k in range(NK):
                    nc.tensor.matmul(qp[:], wq_sb[:, h, k, :],
                                     x_sb[:, c, k, :],
                                     start=(k == 0), stop=(k == NK - 1))
                nc.vector.tensor_copy(qf4[:, h, :], qp[:])

            def u_qrope(c, qf4):
                qs4 = rope_pool.tile([128, HPC, 512], bf16, tag="qs4")
                nc.gpsimd.dma_start(qs4[0:64, :, :], qf4[64:128, :, :])
                nc.gpsimd.dma_start(qs4[64:128, :, :], qf4[0:64, :, :])
                for h in range(HPC):
                    rope_apply(qTs[c][:, h, :], qf4[:, h, :], qs4[:, h, :], c)

            def u_vproj(c):
                """v projection for chunk c: v^T chunk then PE transpose."""
                vtp = psum.tile([128, 512], f32, tag="proj", bufs=2)
                for k in range(NK):
                    nc.tensor.matmul(vtp[:], wv_sb[:, k, :], x_sb[:, c, k, :],
                                     start=(k == 0), stop=(k == NK - 1))
                vt = rope_pool.tile([128, 512], f32, tag="vt")
                nc.vector.tensor_copy(vt[:], vtp[:])
                vtr = psum.tile([128, 512], f32, tag="proj", bufs=2)
                for tt in range(4):
                    nc.tensor.transpose(vtr[:, ts(tt, 128)],
                                        vt[:, ts(tt, 128)], id_sb[:])
                nc.vector.tensor_copy(vs[c][:], vtr[:])

            def u_attn_head(c, h, attn_t):
                nj = 4 * c + 4
                out_ps = psum.tile([128, 512], f32, tag="out", bufs=2)
                pacc = pacc_pool.tile([128, 512], bf16, tag="pacc")
                for j in range(nj):
                    o = j - 4 * c
                    lo = max(o, 0) * 128
                    s_ps = psum.tile([128, 512], f32, tag="s", bufs=3)
                    nc.tensor.matmul(s_ps[:, lo:],
                                     kTs[j // 4][:, ts(j % 4, 128)],
                                     qTs[c][:, h, lo:],
                                     start=True, stop=True)
                    if o >= 0:
                        nc.vector.tensor_add(s_ps[:, lo:lo + 128],
                                             s_ps[:, lo:lo + 128],
                                             mask_sb[:])
                    p = p_pool.tile([128, 512], bf16, tag="p")
                    nc.scalar.activation(p[:, lo:], s_ps[:, lo:], Exp,
                                         bias=0.0, scale=SCALE)
                    if j == 0:
                        nc.vector.tensor_copy(pacc[:], p[:])
                    else:
                        nc.vector.tensor_add(pacc[:, lo:], pacc[:, lo:],
                                             p[:, lo:])
                    nc.tensor.matmul(out_ps[:, lo:],
                                     vs[j // 4][:, ts(j % 4, 128)],
                                     p[:, lo:],
                                     start=(j == 0), stop=(j == nj - 1))
                sums_ps = psum.tile([128, 512], f32, tag="sums", bufs=1)
                nc.tensor.matmul(sums_ps[:], ones_sb[:], pacc[:],
                                 start=True, stop=True)
                rc = recip_pool.tile([128, 512], f32, tag="rc")
                nc.vector.reciprocal_approx_fast(out=rc[:], in_=sums_ps[:])
                nc.vector.tensor_mul(attn_t[:, h, :], out_ps[:], rc[:])

            def u_wo(c, tq, nn, attn_t, y_sb):
                yp = psum.tile([128, 512], f32, tag="s", bufs=3)
                for h in range(HPC):
                    nc.tensor.matmul(yp[:], attn_t[:, h, ts(tq, 128)],
                                     wo_sb[:, h, ts(nn, 512)],
                                     start=(h == 0), stop=(h == 3))
                nc.scalar.copy(y_sb[:, ts(nn, 512)], yp[:])
                row0 = (4 * c + tq) * 128
                if c == NCQ - 1 and tq == 3:
                    # last row-block: store per 512-col slice so the final
                    # DMA isn't serialized behind all four copies
                    nc.sync.dma_start(y_d[row0:row0 + 128, ts(nn, 512)],
                                      y_sb[:, ts(nn, 512)])
                elif nn == 3:
                    nc.sync.dma_start(y_d[row0:row0 + 128, :], y_sb[:])

            # ---- unit construction / schedule ----
            def u_late_loads(c):
                """Deferred bulk loads.  DMA issue is scheduled by readiness,
                so each load's destination is first 'touched' by a tiny DVE
                copy reading this chunk's rope output — a real dependency
                that holds the bulk transfer off the bus until the
                latency-critical rope-swap DMAs of chunks 0/1 are done."""
                if c == 0:
                    nc.vector.tensor_copy(x_sb[:, 2, 0, 0:4], kTs[0][:, 0:4])
                    nc.sync.dma_start(x_sb[:, 2], x_d[:, 2])
                    nc.vector.tensor_copy(tab_sb[:, 2, 0, 0:4],
                                          kTs[0][:, 0:4])
                    nc.sync.dma_start(tab_sb[:, 2], tab_d[:, 2])
                    nc.vector.tensor_copy(wo_sb[:, 0, 0:4],
                                          qTs[0][:, 0, 0:4])
                    nc.sync.dma_start(wo_sb[:], wo_d[:])
                elif c == 1:
                    nc.vector.tensor_copy(x_sb[:, 3, 0, 0:4],
                                          qTs[1][:, 0, 0:4])
                    nc.sync.dma_start(x_sb[:, 3], x_d[:, 3])
                    nc.vector.tensor_copy(tab_sb[:, 3, 0, 0:4],
                                          qTs[1][:, 0, 0:4])
                    nc.sync.dma_start(tab_sb[:, 3], tab_d[:, 3])

            def proj_units(c):
                qf4 = rope_pool.tile([128, HPC, 512], bf16, tag="qf4")
                units = [lambda c=c: u_kproj(c)]
                for h in range(HPC):
                    units.append(lambda c=c, h=h, q=qf4: u_qproj(c, h, q))
                units.append(lambda c=c, q=qf4: u_qrope(c, q))
                units.append(lambda c=c: u_vproj(c))
                if c <= 1:
                    units.append(lambda c=c: u_late_loads(c))
                return units

            def wo_units(c, attn_t):
                units = []
                for tq in range(4):
                    y_sb = y_pool.tile([128, D], bf16, tag="y")
                    for nn in range(4):
                        units.append(
                            lambda c=c, tq=tq, nn=nn, a=attn_t, y=y_sb:
                            u_wo(c, tq, nn, a, y))
                return units

            # chunks 0 and 1 projections up front (fills the load window)
            for u in proj_units(0) + proj_units(1):
                u()
            pending_wo = []
            for c in range(NCQ):
                # interleave next projections + previous chunk's Wo between
                # the attention heads of chunk c
                fillers = (proj_units(c + 2) if c + 2 < NCQ else [])
                fillers += pending_wo
                attn_t = attn_pool.tile([128, HPC, 512], bf16, tag="attnT")
                nf = len(fillers)
                for h in range(HPC):
                    u_attn_head(c, h, attn_t)
                    take = nf // 4 + (1 if h < nf % 4 else 0)
                    for _ in range(take):
                        fillers.pop(0)()
                for u in fillers:
                    u()
                pending_wo = wo_units(c, attn_t)
            for u in pending_wo:
                u()

    nc.finalize()
    return nc


def _prep_inputs(x, Wq, Wk, Wv, Wo, t_len=T):
    """Host-side shard + layout prep -> per-core input maps."""
    import ml_dtypes
    bf16 = ml_dtypes.bfloat16

    NK = D // 128
    NCQ = t_len // 512

    x = np.asarray(x, np.float32)
    Wq = np.asarray(Wq, np.float32)
    Wk = np.asarray(Wk, np.float32)
    Wv = np.asarray(Wv, np.float32)
    Wo = np.asarray(Wo, np.float32)

    # RoPE de-interleave permutation within one head: [evens | odds]
    perm = np.concatenate([np.arange(0, HD, 2), np.arange(1, HD, 2)])

    # rope tables (match reference: freqs = t * base**(-2j/HD))
    inv = 1.0 / (ROPE_BASE ** (np.arange(0, HD, 2, dtype=np.float32) / HD))
    tpos = np.arange(t_len, dtype=np.float32)
    f = inv[:, None] * tpos[None, :]                       # [64, T]
    cos_dup = np.concatenate([np.cos(f), np.cos(f)], 0)    # [128, T]
    ssig = np.concatenate([-np.sin(f), np.sin(f)], 0)      # [128, T]
    # tab[p, c, 0, :] = cos chunk c; tab[p, c, 1, :] = ssig chunk c
    tab = np.stack([cos_dup.reshape(128, NCQ, 512),
                    ssig.reshape(128, NCQ, 512)], axis=2).astype(bf16)

    # strict-lower-triangular causal mask template for the diagonal block
    r = np.arange(128)[:, None]
    col = np.arange(128)[None, :]
    mask_t = np.where(r > col, MASK_VAL, 0.0).astype(bf16)
    ident = np.eye(128, dtype=np.float32)
    jswap = np.roll(np.eye(128, dtype=np.float32), 64, axis=0).astype(bf16)

    in_maps = []
    for b in range(B):
        # xp[p, c, k, j] = x[b, c*512+j, k*128+p]
        xp = np.ascontiguousarray(
            x[b, :t_len].T.reshape(NK, 128, NCQ, 512)
            .transpose(1, 2, 0, 3)).astype(bf16)
        for g in range(KV):
            wq_g = Wq[:, g * DQ:(g + 1) * DQ].reshape(D, HPC, HD)
            wq_g = wq_g[:, :, perm]                       # [D, HPC, 128]
            wqp = np.ascontiguousarray(
                wq_g.reshape(NK, 128, HPC, 128)
                .transpose(1, 2, 0, 3)).astype(bf16)      # [p, h, k, j]
            wk_g = Wk[:, g * HD:(g + 1) * HD][:, perm]
            wkp = np.ascontiguousarray(
                wk_g.reshape(NK, 128, 128).transpose(1, 0, 2)).astype(bf16)
            wv_g = Wv[:, g * HD:(g + 1) * HD]
            wvp = np.ascontiguousarray(
                wv_g.reshape(NK, 128, 128).transpose(1, 0, 2)).astype(bf16)
            # wop[p, h, :] = Wo[(4g+h)*128 + p, :]
            wop = np.ascontiguousarray(
                Wo[g * DQ:(g + 1) * DQ, :].reshape(HPC, 128, D)
                .transpose(1, 0, 2)).astype(bf16)
            in_maps.append({
                "xp": xp, "wq": wqp, "wk": wkp, "wv": wvp,
                "wo": wop, "tab": tab, "mask": mask_t, "ident": ident,
                "jswap": jswap,
            })
    return in_maps


def run(inputs, trace=False, t_len=T):
    """Run the sharded kernel; returns (y_full, BassKernelResults)."""
    from concourse.bass_utils import run_bass_kernel_spmd

    key = ("nc", t_len)
    if key not in _CACHE:
        _CACHE[key] = _build_nc(t_len)
    nc = _CACHE[key]

    in_maps = _prep_inputs(inputs["x"], inputs["Wq"], inputs["Wk"],
                           inputs["Wv"], inputs["Wo"], t_len)
    res = run_bass_kernel_spmd(nc, in_maps, list(range(N_CORES)), trace=trace)

    y = np.empty((B, t_len, D), np.float32)
    for b in range(B):
        acc = np.zeros((t_len, D), np.float32)
        for g in range(KV):
            acc += np.asarray(res.results[b * KV + g]["y"], np.float32)
        y[b] = acc
    return y, res


def kernel(**inputs) -> np.ndarray:
    y, _ = run(inputs, trace=False)
    return y


# revision 49
# speedup vs baseline: 1.0154x; 1.0063x over previous
"""Tensor-parallel GQA multi-head-attention kernel for 8 trn2 NeuronCores.

Problem: B=2, T=2048, D=2048, H=16 q-heads, KV=4 kv-heads, HD=128,
causal attention with interleaved RoPE, y = attn_out @ Wo.

Sharding (tensor-parallel over heads, per the hint):
  core c = b*4 + g   (b = batch index, g = kv-head / q-head-group index)
  Each core computes q-heads 4g..4g+3 and kv-head g for batch b, plus the
  partial output  y_partial = attn_heads @ Wo[rows of those heads]  (row-
  parallel Wo).  The host sums the 4 partials per batch (the unshard of the
  row-parallel all-reduce) and stacks the 2 batches.

On-chip design (per core, bf16 data / f32 PSUM+softmax):
  - host repacks every input into [128, ...] arrays so each load is ONE
    wide dma_start (16KB/partition rows -> 16KB DMA descriptors); x and
    the rope tables are loaded per 512-col chunk so compute starts after
    ~3MB instead of after the full 13MB.
  - projections: q^T[h] = Wq_h^T @ xT per 512-col chunk (PSUM k-accum),
    RoPE'd via half-swap (SBUF-SBUF DMA) + 3 bf16 DVE ops into qT/kT;
    v natural via lhsT = xT tile, 4 T-tiles packed into one PSUM bank.
  - attention per (head, 512-col q chunk): for each 128-row k tile,
    S^T = kT_tile.T @ qT chunk -> PSUM [128,512]; diagonal blocks get a
    -30000 mask add (DVE); ACT computes P = exp(scale*S^T) -> SBUF bf16;
    PV accumulates out^T[HD,512] in PSUM.  Softmax denominators: P tiles
    are accumulated on DVE (bf16) into Pacc, then ONE [128,128]-ones
    matmul broadcasts column sums -> reciprocal -> one DVE multiply.
    Fully-masked (future) blocks are skipped everywhere.
  - Wo: y tile [128,512] = sum_h attnT_h.T @ Wo_h chunk (PSUM), ACT copy
    into a [128, 2048] bf16 staging tile, one dma_start per 128-row block.
  - emission interleaves attention of chunk c with projections of chunk
    c+2 and Wo of chunk c-1 so the PE queue always has independent work
    while exp/rope chains resolve.
"""

import math
import sys

import numpy as np

for _p in ("/opt/trn_rl_repo", "/root/.axon_site",
           "/root/.axon_site/_ro/trn_rl_repo",
           "/root/.axon_site/_ro/pypackages"):
    if _p not in sys.path:
        sys.path.append(_p)

B, T, D = 2, 2048, 2048
H, KV, HD = 16, 4, 128
ROPE_BASE = 10000.0
N_CORES = 8
HPC = 4                  # q heads per core
DQ = HPC * HD            # 512 q-dims per core
SCALE = 1.0 / math.sqrt(HD)
MASK_VAL = -30000.0

_CACHE = {}


def _build_nc(t_len=T):
    """Build the single-core SPMD Bass/Tile program (cached)."""
    import concourse.bass as bass
    import concourse.mybir as mybir
    import concourse.tile as tile
    from concourse import bacc

    f32 = mybir.dt.float32
    bf16 = mybir.dt.bfloat16
    ts = bass.ts

    NT = t_len // 128        # number of 128-row T tiles
    NK = D // 128            # contraction chunks for projections
    NCQ = t_len // 512       # number of 512-wide q chunks

    nc = bacc.Bacc("TRN2", target_bir_lowering=False, debug=False,
                   num_devices=N_CORES)

    x_d = nc.dram_tensor("xp", [128, NCQ, NK, 512], bf16,
                         kind="ExternalInput").ap()
    wq_d = nc.dram_tensor("wq", [128, HPC, NK, 128], bf16,
                          kind="ExternalInput").ap()
    wk_d = nc.dram_tensor("wk", [128, NK, 128], bf16,
                          kind="ExternalInput").ap()
    wv_d = nc.dram_tensor("wv", [128, NK, 128], bf16,
                          kind="ExternalInput").ap()
    wo_d = nc.dram_tensor("wo", [128, HPC, D], bf16,
                          kind="ExternalInput").ap()
    tab_d = nc.dram_tensor("tab", [128, NCQ, 2, 512], bf16,
                           kind="ExternalInput").ap()
    mask_d = nc.dram_tensor("mask", [128, 128], bf16,
                            kind="ExternalInput").ap()
    id_d = nc.dram_tensor("ident", [128, 128], f32,
                          kind="ExternalInput").ap()
    j_d = nc.dram_tensor("jswap", [128, 128], bf16,
                         kind="ExternalInput").ap()
    y_d = nc.dram_tensor("y", [t_len, D], bf16, kind="ExternalOutput").ap()

    Exp = mybir.ActivationFunctionType.Exp

    with tile.TileContext(nc) as tc:
        with (
            tc.tile_pool(name="const", bufs=1) as const,
            tc.tile_pool(name="qkv", bufs=1) as qkv,
            tc.tile_pool(name="attn", bufs=3) as attn_pool,
            tc.tile_pool(name="p", bufs=8) as p_pool,
            tc.tile_pool(name="rope", bufs=2) as rope_pool,
            tc.tile_pool(name="pacc", bufs=2) as pacc_pool,
            tc.tile_pool(name="recip", bufs=2) as recip_pool,
            tc.tile_pool(name="y", bufs=2) as y_pool,
            tc.tile_pool(name="psum", bufs=1, space="PSUM") as psum,
        ):
            # ---- SBUF input tiles ----
            x_sb = const.tile([128, NCQ, NK, 512], bf16, tag="x")
            wq_sb = const.tile([128, HPC, NK, 128], bf16, tag="wq")
            wk_sb = const.tile([128, NK, 128], bf16, tag="wk")
            wv_sb = const.tile([128, NK, 128], bf16, tag="wv")
            wo_sb = const.tile([128, HPC, D], bf16, tag="wo")
            tab_sb = const.tile([128, NCQ, 2, 512], bf16, tag="tab")
            mask_sb = const.tile([128, 128], bf16, tag="mask")
            id_sb = const.tile([128, 128], f32, tag="ident")
            j_sb = const.tile([128, 128], bf16, tag="jswap")
            ones_sb = const.tile([128, 128], bf16, tag="ones")

            # ---- input loads: few wide DMAs, ordered so chunk-0 compute
            # starts as early as possible ----
            # input loads: few wide DMAs, ordered so chunk-0 compute starts
            # as early as possible
            nc.sync.dma_start(wk_sb[:], wk_d[:])
            nc.sync.dma_start(j_sb[:], j_d[:])
            nc.sync.dma_start(x_sb[:, 0, 0:8, :], x_d[:, 0, 0:8, :])
            nc.sync.dma_start(x_sb[:, 0, 8:NK, :], x_d[:, 0, 8:NK, :])
            nc.sync.dma_start(wq_sb[:, 0], wq_d[:, 0])
            nc.sync.dma_start(tab_sb[:, 0], tab_d[:, 0])
            for h in range(1, HPC):
                nc.sync.dma_start(wq_sb[:, h], wq_d[:, h])
            nc.sync.dma_start(wv_sb[:], wv_d[:])
            nc.sync.dma_start(mask_sb[:], mask_d[:])
            nc.sync.dma_start(id_sb[:], id_d[:])
            nc.sync.dma_start(x_sb[:, 1], x_d[:, 1])
            nc.sync.dma_start(tab_sb[:, 1], tab_d[:, 1])
            nc.sync.dma_start(wo_sb[:], wo_d[:])
            for c in range(2, NCQ):
                nc.sync.dma_start(x_sb[:, c], x_d[:, c])
                nc.sync.dma_start(tab_sb[:, c], tab_d[:, c])
            nc.vector.memset(ones_sb[:], 1.0)

            # PE warm-up: serial matmuls on constants during the load window
            # ramp the tensor-engine clock to full speed before real work.
            warm_rhs = const.tile([128, 512], bf16, tag="warm")
            nc.vector.memset(warm_rhs[:], 0.0)
            warm_ps = psum.tile([128, 512], f32, tag="proj", bufs=2)
            for _ in range(24):
                nc.tensor.matmul(warm_ps[:], ones_sb[:], warm_rhs[:],
                                 start=True, stop=True)

            # per-chunk activations (separate tiles keep cross-chunk
            # dependencies precise in the Tile framework)
            qTs = [qkv.tile([128, HPC, 512], bf16, tag=f"qT{c}",
                            name=f"qT{c}") for c in range(NCQ)]
            kTs = [qkv.tile([128, 512], bf16, tag=f"kT{c}",
                            name=f"kT{c}") for c in range(NCQ)]
            vs = [qkv.tile([128, 512], bf16, tag=f"v{c}",
                           name=f"v{c}") for c in range(NCQ)]

            def rope_apply(dst, f, s, c):
                """dst = f*cos + s*ssig for one [128,512] head chunk."""
                tm = rope_pool.tile([128, 512], bf16, tag="tm")
                nc.vector.tensor_mul(dst, f, tab_sb[:, c, 0, :])
                nc.vector.tensor_mul(tm[:], s, tab_sb[:, c, 1, :])
                nc.vector.tensor_add(dst, dst, tm[:])

            def u_kproj(c):
                kp = psum.tile([128, 512], f32, tag="proj", bufs=2)
                for k in range(NK):
                    nc.tensor.matmul(kp[:], wk_sb[:, k, :], x_sb[:, c, k, :],
                                     start=(k == 0), stop=(k == NK - 1))
                kf = rope_pool.tile([128, 512], bf16, tag="kf")
                nc.vector.tensor_copy(kf[:], kp[:])
                if c == 0:
                    # chunk 0's swap is latency-critical and the DMA engines
                    # are saturated with input loads: swap on the PE instead
                    ksp = psum.tile([128, 512], f32, tag="proj", bufs=2)
                    nc.tensor.matmul(ksp[:], j_sb[:], kf[:],
                                     start=True, stop=True)
                    rope_apply(kTs[c][:], kf[:], ksp[:], c)
                else:
                    ks = rope_pool.tile([128, 512], bf16, tag="ks")
                    nc.gpsimd.dma_start(ks[0:64, :], kf[64:128, :])
                    nc.gpsimd.dma_start(ks[64:128, :], kf[0:64, :])
                    rope_apply(kTs[c][:], kf[:], ks[:], c)

            def u_qproj(c, h, qf4):
                qp = psum.tile([128, 512], f32, tag="proj", bufs=2)
                for k in range(NK):
                    nc.tensor.matmul(qp[:], wq_sb[:, h, k, :],
                                     x_sb[:, c, k, :],
                                     start=(k == 0), stop=(k == NK - 1))
                nc.vector.tensor_copy(qf4[:, h, :], qp[:])

            def u_qrope(c, qf4):
                if c == 0:
                    for h in range(HPC):
                        qsp = psum.tile([128, 512], f32, tag="proj", bufs=2)
                        nc.tensor.matmul(qsp[:], j_sb[:], qf4[:, h, :],
                                         start=True, stop=True)
                        rope_apply(qTs[c][:, h, :], qf4[:, h, :], qsp[:], c)
                    return
                qs4 = rope_pool.tile([128, HPC, 512], bf16, tag="qs4")
                nc.gpsimd.dma_start(qs4[0:64, :, :], qf4[64:128, :, :])
                nc.gpsimd.dma_start(qs4[64:128, :, :], qf4[0:64, :, :])
                for h in range(HPC):
                    rope_apply(qTs[c][:, h, :], qf4[:, h, :], qs4[:, h, :], c)

            def u_vproj(c):
                """v projection for chunk c: v^T chunk then PE transpose."""
                vtp = psum.tile([128, 512], f32, tag="proj", bufs=2)
                for k in range(NK):
                    nc.tensor.matmul(vtp[:], wv_sb[:, k, :], x_sb[:, c, k, :],
                                     start=(k == 0), stop=(k == NK - 1))
                vt = rope_pool.tile([128, 512], f32, tag="vt")
                nc.vector.tensor_copy(vt[:], vtp[:])
                vtr = psum.tile([128, 512], f32, tag="proj", bufs=2)
                for tt in range(4):
                    nc.tensor.transpose(vtr[:, ts(tt, 128)],
                                        vt[:, ts(tt, 128)], id_sb[:])
                nc.vector.tensor_copy(vs[c][:], vtr[:])

            def u_attn_head(c, h, attn_t):
                nj = 4 * c + 4
                out_ps = psum.tile([128, 512], f32, tag="out", bufs=2)
                pacc = pacc_pool.tile([128, 512], bf16, tag="pacc")
                for j in range(nj):
                    o = j - 4 * c
                    lo = max(o, 0) * 128
                    s_ps = psum.tile([128, 512], f32, tag="s", bufs=3)
                    nc.tensor.matmul(s_ps[:, lo:],
                                     kTs[j // 4][:, ts(j % 4, 128)],
                                     qTs[c][:, h, lo:],
                                     start=True, stop=True)
                    if o >= 0:
                        nc.vector.tensor_add(s_ps[:, lo:lo + 128],
                                             s_ps[:, lo:lo + 128],
                                             mask_sb[:])
                    p = p_pool.tile([128, 512], bf16, tag="p")
                    nc.scalar.activation(p[:, lo:], s_ps[:, lo:], Exp,
                                         bias=0.0, scale=SCALE)
                    if j == 0:
                        nc.vector.tensor_copy(pacc[:], p[:])
                    else:
                        nc.vector.tensor_add(pacc[:, lo:], pacc[:, lo:],
                                             p[:, lo:])
                    nc.tensor.matmul(out_ps[:, lo:],
                                     vs[j // 4][:, ts(j % 4, 128)],
                                     p[:, lo:],
                                     start=(j == 0), stop=(j == nj - 1))
                sums_ps = psum.tile([128, 512], f32, tag="sums", bufs=1)
                nc.tensor.matmul(sums_ps[:], ones_sb[:], pacc[:],
                                 start=True, stop=True)
                rc = recip_pool.tile([128, 512], f32, tag="rc")
                nc.vector.reciprocal_approx_fast(out=rc[:], in_=sums_ps[:])
                nc.vector.tensor_mul(attn_t[:, h, :], out_ps[:], rc[:])

            def u_wo(c, tq, nn, attn_t, y_sb):
                yp = psum.tile([128, 512], f32, tag="s", bufs=3)
                for h in range(HPC):
                    nc.tensor.matmul(yp[:], attn_t[:, h, ts(tq, 128)],
                                     wo_sb[:, h, ts(nn, 512)],
                                     start=(h == 0), stop=(h == 3))
                nc.scalar.copy(y_sb[:, ts(nn, 512)], yp[:])
                row0 = (4 * c + tq) * 128
                if c == NCQ - 1 and tq == 3:
                    # last row-block: store per 512-col slice so the final
                    # DMA isn't serialized behind all four copies
                    nc.sync.dma_start(y_d[row0:row0 + 128, ts(nn, 512)],
                                      y_sb[:, ts(nn, 512)])
                elif nn == 3:
                    nc.sync.dma_start(y_d[row0:row0 + 128, :], y_sb[:])

            # ---- unit construction / schedule ----
            def proj_units(c):
                qf4 = rope_pool.tile([128, HPC, 512], bf16, tag="qf4")
                units = [lambda c=c: u_kproj(c)]
                for h in range(HPC):
                    units.append(lambda c=c, h=h, q=qf4: u_qproj(c, h, q))
                units.append(lambda c=c, q=qf4: u_qrope(c, q))
                units.append(lambda c=c: u_vproj(c))
                return units

            def wo_units(c, attn_t):
                units = []
                for tq in range(4):
                    y_sb = y_pool.tile([128, D], bf16, tag="y")
                    for nn in range(4):
                        units.append(
                            lambda c=c, tq=tq, nn=nn, a=attn_t, y=y_sb:
                            u_wo(c, tq, nn, a, y))
                return units

            # chunks 0 and 1 projections up front (fills the load window)
            for u in proj_units(0) + proj_units(1):
                u()
            pending_wo = []
            for c in range(NCQ):
                # interleave next projections + previous chunk's Wo between
                # the attention heads of chunk c
                fillers = (proj_units(c + 2) if c + 2 < NCQ else [])
                fillers += pending_wo
                attn_t = attn_pool.tile([128, HPC, 512], bf16, tag="attnT")
                nf = len(fillers)
                for h in range(HPC):
                    u_attn_head(c, h, attn_t)
                    take = nf // 4 + (1 if h < nf % 4 else 0)
                    for _ in range(take):
                        fillers.pop(0)()
                for u in fillers:
                    u()
                pending_wo = wo_units(c, attn_t)
            for u in pending_wo:
                u()

    nc.finalize()
    return nc


def _prep_inputs(x, Wq, Wk, Wv, Wo, t_len=T):
    """Host-side shard + layout prep -> per-core input maps."""
    import ml_dtypes
    bf16 = ml_dtypes.bfloat16

    NK = D // 128
    NCQ = t_len // 512

    x = np.asarray(x, np.float32)
    Wq = np.asarray(Wq, np.float32)
    Wk = np.asarray(Wk, np.float32)
    Wv = np.asarray(Wv, np.float32)
    Wo = np.asarray(Wo, np.float32)

    # RoPE de-interleave permutation within one head: [evens | odds]
    perm = np.concatenate([np.arange(0, HD, 2), np.arange(1, HD, 2)])

    # rope tables (match reference: freqs = t * base**(-2j/HD))
    inv = 1.0 / (ROPE_BASE ** (np.arange(0, HD, 2, dtype=np.float32) / HD))
    tpos = np.arange(t_len, dtype=np.float32)
    f = inv[:, None] * tpos[None, :]                       # [64, T]
    cos_dup = np.concatenate([np.cos(f), np.cos(f)], 0)    # [128, T]
    ssig = np.concatenate([-np.sin(f), np.sin(f)], 0)      # [128, T]
    # tab[p, c, 0, :] = cos chunk c; tab[p, c, 1, :] = ssig chunk c
    tab = np.stack([cos_dup.reshape(128, NCQ, 512),
                    ssig.reshape(128, NCQ, 512)], axis=2).astype(bf16)

    # strict-lower-triangular causal mask template for the diagonal block
    r = np.arange(128)[:, None]
    col = np.arange(128)[None, :]
    mask_t = np.where(r > col, MASK_VAL, 0.0).astype(bf16)
    ident = np.eye(128, dtype=np.float32)
    jswap = np.roll(np.eye(128, dtype=np.float32), 64, axis=0).astype(bf16)

    in_maps = []
    for b in range(B):
        # xp[p, c, k, j] = x[b, c*512+j, k*128+p]
        xp = np.ascontiguousarray(
            x[b, :t_len].T.reshape(NK, 128, NCQ, 512)
            .transpose(1, 2, 0, 3)).astype(bf16)
        for g in range(KV):
            wq_g = Wq[:, g * DQ:(g + 1) * DQ].reshape(D, HPC, HD)
            wq_g = wq_g[:, :, perm]                       # [D, HPC, 128]
            wqp = np.ascontiguousarray(
                wq_g.reshape(NK, 128, HPC, 128)
                .transpose(1, 2, 0, 3)).astype(bf16)      # [p, h, k, j]
            wk_g = Wk[:, g * HD:(g + 1) * HD][:, perm]
            wkp = np.ascontiguousarray(
                wk_g.reshape(NK, 128, 128).transpose(1, 0, 2)).astype(bf16)
            wv_g = Wv[:, g * HD:(g + 1) * HD]
            wvp = np.ascontiguousarray(
                wv_g.reshape(NK, 128, 128).transpose(1, 0, 2)).astype(bf16)
            # wop[p, h, :] = Wo[(4g+h)*128 + p, :]
            wop = np.ascontiguousarray(
                Wo[g * DQ:(g + 1) * DQ, :].reshape(HPC, 128, D)
                .transpose(1, 0, 2)).astype(bf16)
            in_maps.append({
                "xp": xp, "wq": wqp, "wk": wkp, "wv": wvp,
                "wo": wop, "tab": tab, "mask": mask_t, "ident": ident,
                "jswap": jswap,
            })
    return in_maps


def run(inputs, trace=False, t_len=T):
    """Run the sharded kernel; returns (y_full, BassKernelResults)."""
    from concourse.bass_utils import run_bass_kernel_spmd

    key = ("nc", t_len)
    if key not in _CACHE:
        _CACHE[key] = _build_nc(t_len)
    nc = _CACHE[key]

    in_maps = _prep_inputs(inputs["x"], inputs["Wq"], inputs["Wk"],
                           inputs["Wv"], inputs["Wo"], t_len)
    res = run_bass_kernel_spmd(nc, in_maps, list(range(N_CORES)), trace=trace)

    y = np.empty((B, t_len, D), np.float32)
    for b in range(B):
        acc = np.zeros((t_len, D), np.float32)
        for g in range(KV):
            acc += np.asarray(res.results[b * KV + g]["y"], np.float32)
        y[b] = acc
    return y, res


def kernel(**inputs) -> np.ndarray:
    y, _ = run(inputs, trace=False)
    return y


# revision 51
# speedup vs baseline: 1.0610x; 1.0449x over previous
"""Tensor-parallel GQA multi-head-attention kernel for 8 trn2 NeuronCores.

Problem: B=2, T=2048, D=2048, H=16 q-heads, KV=4 kv-heads, HD=128,
causal attention with interleaved RoPE, y = attn_out @ Wo.

Sharding (tensor-parallel over heads, per the hint):
  core c = b*4 + g   (b = batch index, g = kv-head / q-head-group index)
  Each core computes q-heads 4g..4g+3 and kv-head g for batch b, plus the
  partial output  y_partial = attn_heads @ Wo[rows of those heads]  (row-
  parallel Wo).  The host sums the 4 partials per batch (the unshard of the
  row-parallel all-reduce) and stacks the 2 batches.

On-chip design (per core, bf16 data / f32 PSUM+softmax):
  - host repacks every input into [128, ...] arrays so each load is ONE
    wide dma_start (16KB/partition rows -> 16KB DMA descriptors); x and
    the rope tables are loaded per 512-col chunk so compute starts after
    ~3MB instead of after the full 13MB.
  - projections: q^T[h] = Wq_h^T @ xT per 512-col chunk (PSUM k-accum),
    RoPE'd via half-swap (SBUF-SBUF DMA) + 3 bf16 DVE ops into qT/kT;
    v natural via lhsT = xT tile, 4 T-tiles packed into one PSUM bank.
  - attention per (head, 512-col q chunk): for each 128-row k tile,
    S^T = kT_tile.T @ qT chunk -> PSUM [128,512]; diagonal blocks get a
    -30000 mask add (DVE); ACT computes P = exp(scale*S^T) -> SBUF bf16;
    PV accumulates out^T[HD,512] in PSUM.  Softmax denominators: P tiles
    are accumulated on DVE (bf16) into Pacc, then ONE [128,128]-ones
    matmul broadcasts column sums -> reciprocal -> one DVE multiply.
    Fully-masked (future) blocks are skipped everywhere.
  - Wo: y tile [128,512] = sum_h attnT_h.T @ Wo_h chunk (PSUM), ACT copy
    into a [128, 2048] bf16 staging tile, one dma_start per 128-row block.
  - emission interleaves attention of chunk c with projections of chunk
    c+2 and Wo of chunk c-1 so the PE queue always has independent work
    while exp/rope chains resolve.
"""

import math
import sys

import numpy as np

for _p in ("/opt/trn_rl_repo", "/root/.axon_site",
           "/root/.axon_site/_ro/trn_rl_repo",
           "/root/.axon_site/_ro/pypackages"):
    if _p not in sys.path:
        sys.path.append(_p)

B, T, D = 2, 2048, 2048
H, KV, HD = 16, 4, 128
ROPE_BASE = 10000.0
N_CORES = 8
HPC = 4                  # q heads per core
DQ = HPC * HD            # 512 q-dims per core
SCALE = 1.0 / math.sqrt(HD)
MASK_VAL = -30000.0

_CACHE = {}


def _build_nc(t_len=T):
    """Build the single-core SPMD Bass/Tile program (cached)."""
    import concourse.bass as bass
    import concourse.mybir as mybir
    import concourse.tile as tile
    from concourse import bacc

    f32 = mybir.dt.float32
    bf16 = mybir.dt.bfloat16
    ts = bass.ts

    NT = t_len // 128        # number of 128-row T tiles
    NK = D // 128            # contraction chunks for projections
    NCQ = t_len // 512       # number of 512-wide q chunks

    nc = bacc.Bacc("TRN2", target_bir_lowering=False, debug=False,
                   num_devices=N_CORES)

    x_d = nc.dram_tensor("xp", [128, NCQ, NK, 512], bf16,
                         kind="ExternalInput").ap()
    wq_d = nc.dram_tensor("wq", [128, HPC, NK, 128], bf16,
                          kind="ExternalInput").ap()
    wk_d = nc.dram_tensor("wk", [128, NK, 128], bf16,
                          kind="ExternalInput").ap()
    wv_d = nc.dram_tensor("wv", [128, NK, 128], bf16,
                          kind="ExternalInput").ap()
    wo_d = nc.dram_tensor("wo", [128, HPC, D], bf16,
                          kind="ExternalInput").ap()
    tab_d = nc.dram_tensor("tab", [128, NCQ, 2, 512], bf16,
                           kind="ExternalInput").ap()
    mask_d = nc.dram_tensor("mask", [128, 128], bf16,
                            kind="ExternalInput").ap()
    id_d = nc.dram_tensor("ident", [128, 128], f32,
                          kind="ExternalInput").ap()
    j_d = nc.dram_tensor("jswap", [128, 128], bf16,
                         kind="ExternalInput").ap()
    y_d = nc.dram_tensor("y", [t_len, D], bf16, kind="ExternalOutput").ap()

    Exp = mybir.ActivationFunctionType.Exp

    with tile.TileContext(nc) as tc:
        with (
            tc.tile_pool(name="const", bufs=1) as const,
            tc.tile_pool(name="qkv", bufs=1) as qkv,
            tc.tile_pool(name="attn", bufs=3) as attn_pool,
            tc.tile_pool(name="p", bufs=8) as p_pool,
            tc.tile_pool(name="rope", bufs=2) as rope_pool,
            tc.tile_pool(name="pacc", bufs=2) as pacc_pool,
            tc.tile_pool(name="recip", bufs=2) as recip_pool,
            tc.tile_pool(name="y", bufs=2) as y_pool,
            tc.tile_pool(name="psum", bufs=1, space="PSUM") as psum,
        ):
            # ---- SBUF input tiles ----
            x_sb = const.tile([128, NCQ, NK, 512], bf16, tag="x")
            wq_sb = const.tile([128, HPC, NK, 128], bf16, tag="wq")
            wk_sb = const.tile([128, NK, 128], bf16, tag="wk")
            wv_sb = const.tile([128, NK, 128], bf16, tag="wv")
            wo_sb = const.tile([128, HPC, D], bf16, tag="wo")
            tab_sb = const.tile([128, NCQ, 2, 512], bf16, tag="tab")
            mask_sb = const.tile([128, 128], bf16, tag="mask")
            id_sb = const.tile([128, 128], f32, tag="ident")
            j_sb = const.tile([128, 128], bf16, tag="jswap")
            ones_sb = const.tile([128, 128], bf16, tag="ones")

            # ---- input loads: few wide DMAs, ordered so chunk-0 compute
            # starts as early as possible ----
            # input loads: few wide DMAs, ordered so chunk-0 compute starts
            # as early as possible
            nc.sync.dma_start(wk_sb[:], wk_d[:])
            nc.sync.dma_start(j_sb[:], j_d[:])
            nc.sync.dma_start(x_sb[:, 0, 0:8, :], x_d[:, 0, 0:8, :])
            nc.sync.dma_start(x_sb[:, 0, 8:NK, :], x_d[:, 0, 8:NK, :])
            nc.sync.dma_start(wq_sb[:, 0], wq_d[:, 0])
            nc.sync.dma_start(tab_sb[:, 0], tab_d[:, 0])
            for h in range(1, HPC):
                nc.sync.dma_start(wq_sb[:, h], wq_d[:, h])
            nc.sync.dma_start(wv_sb[:], wv_d[:])
            nc.sync.dma_start(mask_sb[:], mask_d[:])
            nc.sync.dma_start(id_sb[:], id_d[:])
            nc.sync.dma_start(x_sb[:, 1], x_d[:, 1])
            nc.sync.dma_start(tab_sb[:, 1], tab_d[:, 1])
            nc.sync.dma_start(wo_sb[:], wo_d[:])
            for c in range(2, NCQ):
                nc.sync.dma_start(x_sb[:, c], x_d[:, c])
                nc.sync.dma_start(tab_sb[:, c], tab_d[:, c])
            nc.vector.memset(ones_sb[:], 1.0)

            # PE warm-up: serial matmuls on constants during the load window
            # ramp the tensor-engine clock to full speed before real work.
            warm_rhs = const.tile([128, 512], bf16, tag="warm")
            nc.vector.memset(warm_rhs[:], 0.0)
            warm_ps = psum.tile([128, 512], f32, tag="proj", bufs=2)
            for _ in range(24):
                nc.tensor.matmul(warm_ps[:], ones_sb[:], warm_rhs[:],
                                 start=True, stop=True)

            # per-chunk activations (separate tiles keep cross-chunk
            # dependencies precise in the Tile framework)
            qTs = [qkv.tile([128, HPC, 512], bf16, tag=f"qT{c}",
                            name=f"qT{c}") for c in range(NCQ)]
            kTs = [qkv.tile([128, 512], bf16, tag=f"kT{c}",
                            name=f"kT{c}") for c in range(NCQ)]
            vs = [qkv.tile([128, 512], bf16, tag=f"v{c}",
                           name=f"v{c}") for c in range(NCQ)]

            def rope_apply(dst, f, s, c):
                """dst = f*cos + s*ssig for one [128,512] head chunk."""
                tm = rope_pool.tile([128, 512], bf16, tag="tm")
                nc.vector.tensor_mul(dst, f, tab_sb[:, c, 0, :])
                nc.vector.tensor_mul(tm[:], s, tab_sb[:, c, 1, :])
                nc.vector.tensor_add(dst, dst, tm[:])

            def u_kproj(c):
                kp = psum.tile([128, 512], f32, tag="proj", bufs=2)
                for k in range(NK):
                    nc.tensor.matmul(kp[:], wk_sb[:, k, :], x_sb[:, c, k, :],
                                     start=(k == 0), stop=(k == NK - 1))
                kf = rope_pool.tile([128, 512], bf16, tag="kf")
                nc.vector.tensor_copy(kf[:], kp[:])
                if c == 0:
                    # chunk 0's swap is latency-critical and the DMA engines
                    # are saturated with input loads: swap on the PE instead
                    ksp = psum.tile([128, 512], f32, tag="proj", bufs=2)
                    nc.tensor.matmul(ksp[:], j_sb[:], kf[:],
                                     start=True, stop=True)
                    rope_apply(kTs[c][:], kf[:], ksp[:], c)
                else:
                    ks = rope_pool.tile([128, 512], bf16, tag="ks")
                    nc.gpsimd.dma_start(ks[0:64, :], kf[64:128, :])
                    nc.gpsimd.dma_start(ks[64:128, :], kf[0:64, :])
                    rope_apply(kTs[c][:], kf[:], ks[:], c)

            def u_qproj(c, h, qf4):
                qp = psum.tile([128, 512], f32, tag="proj", bufs=2)
                for k in range(NK):
                    nc.tensor.matmul(qp[:], wq_sb[:, h, k, :],
                                     x_sb[:, c, k, :],
                                     start=(k == 0), stop=(k == NK - 1))
                nc.vector.tensor_copy(qf4[:, h, :], qp[:])

            def u_qrope(c, qf4):
                if c == 0:
                    for h in range(HPC):
                        qsp = psum.tile([128, 512], f32, tag="proj", bufs=2)
                        nc.tensor.matmul(qsp[:], j_sb[:], qf4[:, h, :],
                                         start=True, stop=True)
                        rope_apply(qTs[c][:, h, :], qf4[:, h, :], qsp[:], c)
                    return
                qs4 = rope_pool.tile([128, HPC, 512], bf16, tag="qs4")
                nc.gpsimd.dma_start(qs4[0:64, :, :], qf4[64:128, :, :])
                nc.gpsimd.dma_start(qs4[64:128, :, :], qf4[0:64, :, :])
                for h in range(HPC):
                    rope_apply(qTs[c][:, h, :], qf4[:, h, :], qs4[:, h, :], c)

            def u_vproj(c):
                """v projection for chunk c: v^T chunk then PE transpose."""
                vtp = psum.tile([128, 512], f32, tag="proj", bufs=2)
                for k in range(NK):
                    nc.tensor.matmul(vtp[:], wv_sb[:, k, :], x_sb[:, c, k, :],
                                     start=(k == 0), stop=(k == NK - 1))
                vt = rope_pool.tile([128, 512], f32, tag="vt")
                nc.vector.tensor_copy(vt[:], vtp[:])
                vtr = psum.tile([128, 512], f32, tag="proj", bufs=2)
                for tt in range(4):
                    nc.tensor.transpose(vtr[:, ts(tt, 128)],
                                        vt[:, ts(tt, 128)], id_sb[:])
                nc.vector.tensor_copy(vs[c][:], vtr[:])

            def u_attn_pair(c, h0, attn_t):
                """Attention for heads h0, h0+1 with interleaved j-loops:
                the second head's S matmul hides the first head's exp
                latency so the PE never waits on the Scalar engine."""
                nj = 4 * c + 4
                heads = (h0, h0 + 1)
                out = {}
                pacc = {}
                for h in heads:
                    out[h] = psum.tile([128, 512], f32, tag="out", bufs=2,
                                       name=f"out{h}")
                    pacc[h] = pacc_pool.tile([128, 512], bf16, tag="pacc",
                                             name=f"pacc{h}")

                def s_exp(j, h, lo):
                    s_ps = psum.tile([128, 512], f32, tag="s", bufs=3)
                    nc.tensor.matmul(s_ps[:, lo:],
                                     kTs[j // 4][:, ts(j % 4, 128)],
                                     qTs[c][:, h, lo:],
                                     start=True, stop=True)
                    if lo or j == 4 * c:
                        nc.vector.tensor_add(s_ps[:, lo:lo + 128],
                                             s_ps[:, lo:lo + 128],
                                             mask_sb[:])
                    p = p_pool.tile([128, 512], bf16, tag="p")
                    nc.scalar.activation(p[:, lo:], s_ps[:, lo:], Exp,
                                         bias=0.0, scale=SCALE)
                    if j == 0:
                        nc.vector.tensor_copy(pacc[h][:], p[:])
                    else:
                        nc.vector.tensor_add(pacc[h][:, lo:],
                                             pacc[h][:, lo:], p[:, lo:])
                    return p

                for j in range(nj):
                    o = j - 4 * c
                    lo = max(o, 0) * 128
                    ps = [s_exp(j, h, lo) for h in heads]
                    for h, p in zip(heads, ps):
                        nc.tensor.matmul(out[h][:, lo:],
                                         vs[j // 4][:, ts(j % 4, 128)],
                                         p[:, lo:],
                                         start=(j == 0), stop=(j == nj - 1))
                for h in heads:
                    sums_ps = psum.tile([128, 512], f32, tag="sums", bufs=1)
                    nc.tensor.matmul(sums_ps[:], ones_sb[:], pacc[h][:],
                                     start=True, stop=True)
                    rc = recip_pool.tile([128, 512], f32, tag="rc")
                    nc.vector.reciprocal_approx_fast(out=rc[:],
                                                     in_=sums_ps[:])
                    nc.vector.tensor_mul(attn_t[:, h, :], out[h][:], rc[:])

            def u_wo(c, tq, nn, attn_t, y_sb):
                yp = psum.tile([128, 512], f32, tag="s", bufs=3)
                for h in range(HPC):
                    nc.tensor.matmul(yp[:], attn_t[:, h, ts(tq, 128)],
                                     wo_sb[:, h, ts(nn, 512)],
                                     start=(h == 0), stop=(h == 3))
                nc.scalar.copy(y_sb[:, ts(nn, 512)], yp[:])
                row0 = (4 * c + tq) * 128
                if c == NCQ - 1 and tq == 3:
                    # last row-block: store per 512-col slice so the final
                    # DMA isn't serialized behind all four copies
                    nc.sync.dma_start(y_d[row0:row0 + 128, ts(nn, 512)],
                                      y_sb[:, ts(nn, 512)])
                elif nn == 3:
                    nc.sync.dma_start(y_d[row0:row0 + 128, :], y_sb[:])

            # ---- unit construction / schedule ----
            def proj_units(c):
                qf4 = rope_pool.tile([128, HPC, 512], bf16, tag="qf4")
                units = [lambda c=c: u_kproj(c)]
                for h in range(HPC):
                    units.append(lambda c=c, h=h, q=qf4: u_qproj(c, h, q))
                units.append(lambda c=c, q=qf4: u_qrope(c, q))
                units.append(lambda c=c: u_vproj(c))
                return units

            def wo_units(c, attn_t):
                units = []
                for tq in range(4):
                    y_sb = y_pool.tile([128, D], bf16, tag="y")
                    for nn in range(4):
                        units.append(
                            lambda c=c, tq=tq, nn=nn, a=attn_t, y=y_sb:
                            u_wo(c, tq, nn, a, y))
                return units

            # chunks 0 and 1 projections up front (fills the load window)
            for u in proj_units(0) + proj_units(1):
                u()
            pending_wo = []
            for c in range(NCQ):
                # interleave next projections + previous chunk's Wo between
                # the attention heads of chunk c
                fillers = (proj_units(c + 2) if c + 2 < NCQ else [])
                fillers += pending_wo
                attn_t = attn_pool.tile([128, HPC, 512], bf16, tag="attnT")
                nf = len(fillers)
                for i, h0 in enumerate((0, 2)):
                    u_attn_pair(c, h0, attn_t)
                    take = nf // 2 + (1 if i < nf % 2 else 0)
                    for _ in range(take):
                        fillers.pop(0)()
                for u in fillers:
                    u()
                pending_wo = wo_units(c, attn_t)
            for u in pending_wo:
                u()

    nc.finalize()
    return nc


def _prep_inputs(x, Wq, Wk, Wv, Wo, t_len=T):
    """Host-side shard + layout prep -> per-core input maps."""
    import ml_dtypes
    bf16 = ml_dtypes.bfloat16

    NK = D // 128
    NCQ = t_len // 512

    x = np.asarray(x, np.float32)
    Wq = np.asarray(Wq, np.float32)
    Wk = np.asarray(Wk, np.float32)
    Wv = np.asarray(Wv, np.float32)
    Wo = np.asarray(Wo, np.float32)

    # RoPE de-interleave permutation within one head: [evens | odds]
    perm = np.concatenate([np.arange(0, HD, 2), np.arange(1, HD, 2)])

    # rope tables (match reference: freqs = t * base**(-2j/HD))
    inv = 1.0 / (ROPE_BASE ** (np.arange(0, HD, 2, dtype=np.float32) / HD))
    tpos = np.arange(t_len, dtype=np.float32)
    f = inv[:, None] * tpos[None, :]                       # [64, T]
    cos_dup = np.concatenate([np.cos(f), np.cos(f)], 0)    # [128, T]
    ssig = np.concatenate([-np.sin(f), np.sin(f)], 0)      # [128, T]
    # tab[p, c, 0, :] = cos chunk c; tab[p, c, 1, :] = ssig chunk c
    tab = np.stack([cos_dup.reshape(128, NCQ, 512),
                    ssig.reshape(128, NCQ, 512)], axis=2).astype(bf16)

    # strict-lower-triangular causal mask template for the diagonal block
    r = np.arange(128)[:, None]
    col = np.arange(128)[None, :]
    mask_t = np.where(r > col, MASK_VAL, 0.0).astype(bf16)
    ident = np.eye(128, dtype=np.float32)
    jswap = np.roll(np.eye(128, dtype=np.float32), 64, axis=0).astype(bf16)

    in_maps = []
    for b in range(B):
        # xp[p, c, k, j] = x[b, c*512+j, k*128+p]
        xp = np.ascontiguousarray(
            x[b, :t_len].T.reshape(NK, 128, NCQ, 512)
            .transpose(1, 2, 0, 3)).astype(bf16)
        for g in range(KV):
            wq_g = Wq[:, g * DQ:(g + 1) * DQ].reshape(D, HPC, HD)
            wq_g = wq_g[:, :, perm]                       # [D, HPC, 128]
            wqp = np.ascontiguousarray(
                wq_g.reshape(NK, 128, HPC, 128)
                .transpose(1, 2, 0, 3)).astype(bf16)      # [p, h, k, j]
            wk_g = Wk[:, g * HD:(g + 1) * HD][:, perm]
            wkp = np.ascontiguousarray(
                wk_g.reshape(NK, 128, 128).transpose(1, 0, 2)).astype(bf16)
            wv_g = Wv[:, g * HD:(g + 1) * HD]
            wvp = np.ascontiguousarray(
                wv_g.reshape(NK, 128, 128).transpose(1, 0, 2)).astype(bf16)
            # wop[p, h, :] = Wo[(4g+h)*128 + p, :]
            wop = np.ascontiguousarray(
                Wo[g * DQ:(g + 1) * DQ, :].reshape(HPC, 128, D)
                .transpose(1, 0, 2)).astype(bf16)
            in_maps.append({
                "xp": xp, "wq": wqp, "wk": wkp, "wv": wvp,
                "wo": wop, "tab": tab, "mask": mask_t, "ident": ident,
                "jswap": jswap,
            })
    return in_maps


def run(inputs, trace=False, t_len=T):
    """Run the sharded kernel; returns (y_full, BassKernelResults)."""
    from concourse.bass_utils import run_bass_kernel_spmd

    key = ("nc", t_len)
    if key not in _CACHE:
        _CACHE[key] = _build_nc(t_len)
    nc = _CACHE[key]

    in_maps = _prep_inputs(inputs["x"], inputs["Wq"], inputs["Wk"],
                           inputs["Wv"], inputs["Wo"], t_len)
    res = run_bass_kernel_spmd(nc, in_maps, list(range(N_CORES)), trace=trace)

    y = np.empty((B, t_len, D), np.float32)
    for b in range(B):
        acc = np.zeros((t_len, D), np.float32)
        for g in range(KV):
            acc += np.asarray(res.results[b * KV + g]["y"], np.float32)
        y[b] = acc
    return y, res


def kernel(**inputs) -> np.ndarray:
    y, _ = run(inputs, trace=False)
    return y


# revision 52
# speedup vs baseline: 1.0611x; 1.0001x over previous
"""Tensor-parallel GQA multi-head-attention kernel for 8 trn2 NeuronCores.

Problem: B=2, T=2048, D=2048, H=16 q-heads, KV=4 kv-heads, HD=128,
causal attention with interleaved RoPE, y = attn_out @ Wo.

Sharding (tensor-parallel over heads, per the hint):
  core c = b*4 + g   (b = batch index, g = kv-head / q-head-group index)
  Each core computes q-heads 4g..4g+3 and kv-head g for batch b, plus the
  partial output  y_partial = attn_heads @ Wo[rows of those heads]  (row-
  parallel Wo).  The host sums the 4 partials per batch (the unshard of the
  row-parallel all-reduce) and stacks the 2 batches.

On-chip design (per core, bf16 data / f32 PSUM+softmax):
  - host repacks every input into [128, ...] arrays so each load is ONE
    wide dma_start (16KB/partition rows -> 16KB DMA descriptors); x and
    the rope tables are loaded per 512-col chunk so compute starts after
    ~3MB instead of after the full 13MB.
  - projections: q^T[h] = Wq_h^T @ xT per 512-col chunk (PSUM k-accum),
    RoPE'd via half-swap (SBUF-SBUF DMA) + 3 bf16 DVE ops into qT/kT;
    v natural via lhsT = xT tile, 4 T-tiles packed into one PSUM bank.
  - attention per (head, 512-col q chunk): for each 128-row k tile,
    S^T = kT_tile.T @ qT chunk -> PSUM [128,512]; diagonal blocks get a
    -30000 mask add (DVE); ACT computes P = exp(scale*S^T) -> SBUF bf16;
    PV accumulates out^T[HD,512] in PSUM.  Softmax denominators: P tiles
    are accumulated on DVE (bf16) into Pacc, then ONE [128,128]-ones
    matmul broadcasts column sums -> reciprocal -> one DVE multiply.
    Fully-masked (future) blocks are skipped everywhere.
  - Wo: y tile [128,512] = sum_h attnT_h.T @ Wo_h chunk (PSUM), ACT copy
    into a [128, 2048] bf16 staging tile, one dma_start per 128-row block.
  - emission interleaves attention of chunk c with projections of chunk
    c+2 and Wo of chunk c-1 so the PE queue always has independent work
    while exp/rope chains resolve.
"""

import math
import sys

import numpy as np

for _p in ("/opt/trn_rl_repo", "/root/.axon_site",
           "/root/.axon_site/_ro/trn_rl_repo",
           "/root/.axon_site/_ro/pypackages"):
    if _p not in sys.path:
        sys.path.append(_p)

B, T, D = 2, 2048, 2048
H, KV, HD = 16, 4, 128
ROPE_BASE = 10000.0
N_CORES = 8
HPC = 4                  # q heads per core
DQ = HPC * HD            # 512 q-dims per core
SCALE = 1.0 / math.sqrt(HD)
MASK_VAL = -30000.0

_CACHE = {}


def _build_nc(t_len=T):
    """Build the single-core SPMD Bass/Tile program (cached)."""
    import concourse.bass as bass
    import concourse.mybir as mybir
    import concourse.tile as tile
    from concourse import bacc

    f32 = mybir.dt.float32
    bf16 = mybir.dt.bfloat16
    ts = bass.ts

    NT = t_len // 128        # number of 128-row T tiles
    NK = D // 128            # contraction chunks for projections
    NCQ = t_len // 512       # number of 512-wide q chunks

    nc = bacc.Bacc("TRN2", target_bir_lowering=False, debug=False,
                   num_devices=N_CORES)

    x_d = nc.dram_tensor("xp", [128, NCQ, NK, 512], bf16,
                         kind="ExternalInput").ap()
    wq_d = nc.dram_tensor("wq", [128, HPC, NK, 128], bf16,
                          kind="ExternalInput").ap()
    wk_d = nc.dram_tensor("wk", [128, NK, 128], bf16,
                          kind="ExternalInput").ap()
    wv_d = nc.dram_tensor("wv", [128, NK, 128], bf16,
                          kind="ExternalInput").ap()
    wo_d = nc.dram_tensor("wo", [128, HPC, D], bf16,
                          kind="ExternalInput").ap()
    tab_d = nc.dram_tensor("tab", [128, NCQ, 2, 512], bf16,
                           kind="ExternalInput").ap()
    mask_d = nc.dram_tensor("mask", [128, 128], bf16,
                            kind="ExternalInput").ap()
    id_d = nc.dram_tensor("ident", [128, 128], f32,
                          kind="ExternalInput").ap()
    j_d = nc.dram_tensor("jswap", [128, 128], bf16,
                         kind="ExternalInput").ap()
    y_d = nc.dram_tensor("y", [t_len, D], bf16, kind="ExternalOutput").ap()

    Exp = mybir.ActivationFunctionType.Exp

    with tile.TileContext(nc) as tc:
        with (
            tc.tile_pool(name="const", bufs=1) as const,
            tc.tile_pool(name="qkv", bufs=1) as qkv,
            tc.tile_pool(name="attn", bufs=3) as attn_pool,
            tc.tile_pool(name="p", bufs=8) as p_pool,
            tc.tile_pool(name="rope", bufs=2) as rope_pool,
            tc.tile_pool(name="pacc", bufs=2) as pacc_pool,
            tc.tile_pool(name="recip", bufs=2) as recip_pool,
            tc.tile_pool(name="y", bufs=2) as y_pool,
            tc.tile_pool(name="psum", bufs=1, space="PSUM") as psum,
        ):
            # ---- SBUF input tiles ----
            x_sb = const.tile([128, NCQ, NK, 512], bf16, tag="x")
            wq_sb = const.tile([128, HPC, NK, 128], bf16, tag="wq")
            wk_sb = const.tile([128, NK, 128], bf16, tag="wk")
            wv_sb = const.tile([128, NK, 128], bf16, tag="wv")
            wo_sb = const.tile([128, HPC, D], bf16, tag="wo")
            tab_sb = const.tile([128, NCQ, 2, 512], bf16, tag="tab")
            mask_sb = const.tile([128, 128], bf16, tag="mask")
            id_sb = const.tile([128, 128], f32, tag="ident")
            j_sb = const.tile([128, 128], bf16, tag="jswap")
            ones_sb = const.tile([128, 128], bf16, tag="ones")

            # ---- input loads: few wide DMAs, ordered so chunk-0 compute
            # starts as early as possible ----
            # input loads: few wide DMAs, ordered so chunk-0 compute starts
            # as early as possible
            nc.sync.dma_start(wk_sb[:], wk_d[:])
            nc.sync.dma_start(j_sb[:], j_d[:])
            nc.sync.dma_start(x_sb[:, 0, 0:8, :], x_d[:, 0, 0:8, :])
            nc.sync.dma_start(x_sb[:, 0, 8:NK, :], x_d[:, 0, 8:NK, :])
            nc.sync.dma_start(wq_sb[:, 0], wq_d[:, 0])
            nc.sync.dma_start(tab_sb[:, 0], tab_d[:, 0])
            for h in range(1, HPC):
                nc.sync.dma_start(wq_sb[:, h], wq_d[:, h])
            nc.sync.dma_start(wv_sb[:], wv_d[:])
            nc.sync.dma_start(mask_sb[:], mask_d[:])
            nc.sync.dma_start(id_sb[:], id_d[:])
            nc.sync.dma_start(x_sb[:, 1], x_d[:, 1])
            nc.sync.dma_start(tab_sb[:, 1], tab_d[:, 1])
            nc.sync.dma_start(wo_sb[:], wo_d[:])
            for c in range(2, NCQ):
                nc.sync.dma_start(x_sb[:, c], x_d[:, c])
                nc.sync.dma_start(tab_sb[:, c], tab_d[:, c])
            nc.vector.memset(ones_sb[:], 1.0)

            # PE warm-up: serial matmuls on constants during the load window
            # ramp the tensor-engine clock to full speed before real work.
            warm_rhs = const.tile([128, 512], bf16, tag="warm")
            nc.vector.memset(warm_rhs[:], 0.0)
            warm_ps = psum.tile([128, 512], f32, tag="proj", bufs=2)
            for _ in range(24):
                nc.tensor.matmul(warm_ps[:], ones_sb[:], warm_rhs[:],
                                 start=True, stop=True)

            # per-chunk activations (separate tiles keep cross-chunk
            # dependencies precise in the Tile framework)
            qTs = [qkv.tile([128, HPC, 512], bf16, tag=f"qT{c}",
                            name=f"qT{c}") for c in range(NCQ)]
            kTs = [qkv.tile([128, 512], bf16, tag=f"kT{c}",
                            name=f"kT{c}") for c in range(NCQ)]
            vs = [qkv.tile([128, 512], bf16, tag=f"v{c}",
                           name=f"v{c}") for c in range(NCQ)]

            def rope_apply(dst, f, s, c):
                """dst = f*cos + s*ssig for one [128,512] head chunk."""
                tm = rope_pool.tile([128, 512], bf16, tag="tm")
                nc.vector.tensor_mul(dst, f, tab_sb[:, c, 0, :])
                nc.vector.tensor_mul(tm[:], s, tab_sb[:, c, 1, :])
                nc.vector.tensor_add(dst, dst, tm[:])

            def u_kproj(c):
                kp = psum.tile([128, 512], f32, tag="proj", bufs=2)
                for k in range(NK):
                    nc.tensor.matmul(kp[:], wk_sb[:, k, :], x_sb[:, c, k, :],
                                     start=(k == 0), stop=(k == NK - 1))
                kf = rope_pool.tile([128, 512], bf16, tag="kf")
                nc.vector.tensor_copy(kf[:], kp[:])
                if c == 0:
                    # chunk 0's swap is latency-critical and the DMA engines
                    # are saturated with input loads: swap on the PE instead
                    ksp = psum.tile([128, 512], f32, tag="proj", bufs=2)
                    nc.tensor.matmul(ksp[:], j_sb[:], kf[:],
                                     start=True, stop=True)
                    rope_apply(kTs[c][:], kf[:], ksp[:], c)
                else:
                    ks = rope_pool.tile([128, 512], bf16, tag="ks")
                    nc.gpsimd.dma_start(ks[0:64, :], kf[64:128, :])
                    nc.gpsimd.dma_start(ks[64:128, :], kf[0:64, :])
                    rope_apply(kTs[c][:], kf[:], ks[:], c)

            def u_qproj(c, h, qf4):
                qp = psum.tile([128, 512], f32, tag="proj", bufs=2)
                for k in range(NK):
                    nc.tensor.matmul(qp[:], wq_sb[:, h, k, :],
                                     x_sb[:, c, k, :],
                                     start=(k == 0), stop=(k == NK - 1))
                nc.vector.tensor_copy(qf4[:, h, :], qp[:])

            def u_qrope(c, qf4):
                if c == 0:
                    for h in range(HPC):
                        qsp = psum.tile([128, 512], f32, tag="proj", bufs=2)
                        nc.tensor.matmul(qsp[:], j_sb[:], qf4[:, h, :],
                                         start=True, stop=True)
                        rope_apply(qTs[c][:, h, :], qf4[:, h, :], qsp[:], c)
                    return
                qs4 = rope_pool.tile([128, HPC, 512], bf16, tag="qs4")
                nc.gpsimd.dma_start(qs4[0:64, :, :], qf4[64:128, :, :])
                nc.gpsimd.dma_start(qs4[64:128, :, :], qf4[0:64, :, :])
                for h in range(HPC):
                    rope_apply(qTs[c][:, h, :], qf4[:, h, :], qs4[:, h, :], c)

            def u_vproj(c):
                """v projection for chunk c: v^T chunk then PE transpose."""
                vtp = psum.tile([128, 512], f32, tag="proj", bufs=2)
                for k in range(NK):
                    nc.tensor.matmul(vtp[:], wv_sb[:, k, :], x_sb[:, c, k, :],
                                     start=(k == 0), stop=(k == NK - 1))
                vt = rope_pool.tile([128, 512], f32, tag="vt")
                nc.vector.tensor_copy(vt[:], vtp[:])
                vtr = psum.tile([128, 512], f32, tag="proj", bufs=2)
                for tt in range(4):
                    nc.tensor.transpose(vtr[:, ts(tt, 128)],
                                        vt[:, ts(tt, 128)], id_sb[:])
                nc.vector.tensor_copy(vs[c][:], vtr[:])

            def u_attn_pair(c, h0, attn_t):
                """Attention for heads h0, h0+1 with interleaved j-loops:
                the second head's S matmul hides the first head's exp
                latency so the PE never waits on the Scalar engine."""
                nj = 4 * c + 4
                heads = (h0, h0 + 1)
                out = {}
                pacc = {}
                for h in heads:
                    out[h] = psum.tile([128, 512], f32, tag="out", bufs=2,
                                       name=f"out{h}")
                    pacc[h] = pacc_pool.tile([128, 512], bf16, tag="pacc",
                                             name=f"pacc{h}")

                def s_exp(j, h, lo):
                    s_ps = psum.tile([128, 512], f32, tag="s", bufs=3)
                    nc.tensor.matmul(s_ps[:, lo:],
                                     kTs[j // 4][:, ts(j % 4, 128)],
                                     qTs[c][:, h, lo:],
                                     start=True, stop=True)
                    if lo or j == 4 * c:
                        nc.vector.tensor_add(s_ps[:, lo:lo + 128],
                                             s_ps[:, lo:lo + 128],
                                             mask_sb[:])
                    p = p_pool.tile([128, 512], bf16, tag="p")
                    nc.scalar.activation(p[:, lo:], s_ps[:, lo:], Exp,
                                         bias=0.0, scale=SCALE)
                    if j == 0:
                        nc.vector.tensor_copy(pacc[h][:], p[:])
                    else:
                        nc.vector.tensor_add(pacc[h][:, lo:],
                                             pacc[h][:, lo:], p[:, lo:])
                    return p

                for j in range(nj):
                    o = j - 4 * c
                    lo = max(o, 0) * 128
                    ps = [s_exp(j, h, lo) for h in heads]
                    for h, p in zip(heads, ps):
                        nc.tensor.matmul(out[h][:, lo:],
                                         vs[j // 4][:, ts(j % 4, 128)],
                                         p[:, lo:],
                                         start=(j == 0), stop=(j == nj - 1))
                for h in heads:
                    sums_ps = psum.tile([128, 512], f32, tag="sums", bufs=1)
                    nc.tensor.matmul(sums_ps[:], ones_sb[:], pacc[h][:],
                                     start=True, stop=True)
                    rc = recip_pool.tile([128, 512], f32, tag="rc")
                    nc.vector.reciprocal_approx_fast(out=rc[:],
                                                     in_=sums_ps[:])
                    nc.vector.tensor_mul(attn_t[:, h, :], out[h][:], rc[:])

            def u_wo(c, tq, nn, attn_t, y_sb):
                yp = psum.tile([128, 512], f32, tag="s", bufs=3)
                for h in range(HPC):
                    nc.tensor.matmul(yp[:], attn_t[:, h, ts(tq, 128)],
                                     wo_sb[:, h, ts(nn, 512)],
                                     start=(h == 0), stop=(h == 3))
                nc.scalar.copy(y_sb[:, ts(nn, 512)], yp[:])
                row0 = (4 * c + tq) * 128
                if c == NCQ - 1 and tq == 3:
                    # last row-block: store per 512-col slice so the final
                    # DMA isn't serialized behind all four copies
                    nc.sync.dma_start(y_d[row0:row0 + 128, ts(nn, 512)],
                                      y_sb[:, ts(nn, 512)])
                elif nn == 3:
                    nc.sync.dma_start(y_d[row0:row0 + 128, :], y_sb[:])

            # ---- unit construction / schedule ----
            def proj_units(c):
                qf4 = rope_pool.tile([128, HPC, 512], bf16, tag="qf4")
                units = [lambda c=c: u_kproj(c)]
                for h in range(HPC):
                    units.append(lambda c=c, h=h, q=qf4: u_qproj(c, h, q))
                units.append(lambda c=c: u_vproj(c))
                units.append(lambda c=c, q=qf4: u_qrope(c, q))
                return units

            def wo_units(c, attn_t):
                units = []
                for tq in range(4):
                    y_sb = y_pool.tile([128, D], bf16, tag="y")
                    for nn in range(4):
                        units.append(
                            lambda c=c, tq=tq, nn=nn, a=attn_t, y=y_sb:
                            u_wo(c, tq, nn, a, y))
                return units

            # chunks 0 and 1 projections up front (fills the load window)
            for u in proj_units(0) + proj_units(1):
                u()
            pending_wo = []
            for c in range(NCQ):
                # interleave next projections + previous chunk's Wo between
                # the attention heads of chunk c
                fillers = (proj_units(c + 2) if c + 2 < NCQ else [])
                fillers += pending_wo
                attn_t = attn_pool.tile([128, HPC, 512], bf16, tag="attnT")
                nf = len(fillers)
                for i, h0 in enumerate((0, 2)):
                    u_attn_pair(c, h0, attn_t)
                    take = nf // 2 + (1 if i < nf % 2 else 0)
                    for _ in range(take):
                        fillers.pop(0)()
                for u in fillers:
                    u()
                pending_wo = wo_units(c, attn_t)
            for u in pending_wo:
                u()

    nc.finalize()
    return nc


def _prep_inputs(x, Wq, Wk, Wv, Wo, t_len=T):
    """Host-side shard + layout prep -> per-core input maps."""
    import ml_dtypes
    bf16 = ml_dtypes.bfloat16

    NK = D // 128
    NCQ = t_len // 512

    x = np.asarray(x, np.float32)
    Wq = np.asarray(Wq, np.float32)
    Wk = np.asarray(Wk, np.float32)
    Wv = np.asarray(Wv, np.float32)
    Wo = np.asarray(Wo, np.float32)

    # RoPE de-interleave permutation within one head: [evens | odds]
    perm = np.concatenate([np.arange(0, HD, 2), np.arange(1, HD, 2)])

    # rope tables (match reference: freqs = t * base**(-2j/HD))
    inv = 1.0 / (ROPE_BASE ** (np.arange(0, HD, 2, dtype=np.float32) / HD))
    tpos = np.arange(t_len, dtype=np.float32)
    f = inv[:, None] * tpos[None, :]                       # [64, T]
    cos_dup = np.concatenate([np.cos(f), np.cos(f)], 0)    # [128, T]
    ssig = np.concatenate([-np.sin(f), np.sin(f)], 0)      # [128, T]
    # tab[p, c, 0, :] = cos chunk c; tab[p, c, 1, :] = ssig chunk c
    tab = np.stack([cos_dup.reshape(128, NCQ, 512),
                    ssig.reshape(128, NCQ, 512)], axis=2).astype(bf16)

    # strict-lower-triangular causal mask template for the diagonal block
    r = np.arange(128)[:, None]
    col = np.arange(128)[None, :]
    mask_t = np.where(r > col, MASK_VAL, 0.0).astype(bf16)
    ident = np.eye(128, dtype=np.float32)
    jswap = np.roll(np.eye(128, dtype=np.float32), 64, axis=0).astype(bf16)

    in_maps = []
    for b in range(B):
        # xp[p, c, k, j] = x[b, c*512+j, k*128+p]
        xp = np.ascontiguousarray(
            x[b, :t_len].T.reshape(NK, 128, NCQ, 512)
            .transpose(1, 2, 0, 3)).astype(bf16)
        for g in range(KV):
            wq_g = Wq[:, g * DQ:(g + 1) * DQ].reshape(D, HPC, HD)
            wq_g = wq_g[:, :, perm]                       # [D, HPC, 128]
            wqp = np.ascontiguousarray(
                wq_g.reshape(NK, 128, HPC, 128)
                .transpose(1, 2, 0, 3)).astype(bf16)      # [p, h, k, j]
            wk_g = Wk[:, g * HD:(g + 1) * HD][:, perm]
            wkp = np.ascontiguousarray(
                wk_g.reshape(NK, 128, 128).transpose(1, 0, 2)).astype(bf16)
            wv_g = Wv[:, g * HD:(g + 1) * HD]
            wvp = np.ascontiguousarray(
                wv_g.reshape(NK, 128, 128).transpose(1, 0, 2)).astype(bf16)
            # wop[p, h, :] = Wo[(4g+h)*128 + p, :]
            wop = np.ascontiguousarray(
                Wo[g * DQ:(g + 1) * DQ, :].reshape(HPC, 128, D)
                .transpose(1, 0, 2)).astype(bf16)
            in_maps.append({
                "xp": xp, "wq": wqp, "wk": wkp, "wv": wvp,
                "wo": wop, "tab": tab, "mask": mask_t, "ident": ident,
                "jswap": jswap,
            })
    return in_maps


def run(inputs, trace=False, t_len=T):
    """Run the sharded kernel; returns (y_full, BassKernelResults)."""
    from concourse.bass_utils import run_bass_kernel_spmd

    key = ("nc", t_len)
    if key not in _CACHE:
        _CACHE[key] = _build_nc(t_len)
    nc = _CACHE[key]

    in_maps = _prep_inputs(inputs["x"], inputs["Wq"], inputs["Wk"],
                           inputs["Wv"], inputs["Wo"], t_len)
    res = run_bass_kernel_spmd(nc, in_maps, list(range(N_CORES)), trace=trace)

    y = np.empty((B, t_len, D), np.float32)
    for b in range(B):
        acc = np.zeros((t_len, D), np.float32)
        for g in range(KV):
            acc += np.asarray(res.results[b * KV + g]["y"], np.float32)
        y[b] = acc
    return y, res


def kernel(**inputs) -> np.ndarray:
    y, _ = run(inputs, trace=False)
    return y
